# revision 1
# baseline (speedup 1.0000x reference)
"""CrossAttention (single-head) Trainium2 kernel, 8-core data-parallel.

Full inputs in, full output out. Internally: batch 16 is sharded 2-per-core
across 8 NeuronCores; each core runs the whole attention layer for its two
batches in bf16 (f32 PSUM accumulation), with activations kept in transposed
[d, s] layout so every matmul contracts over the partition dim without any
on-chip transposes of large tensors.
"""

import sys

sys.path.insert(0, "/opt/trn_rl_repo")

import numpy as np
import ml_dtypes

import concourse.bass as bass
import concourse.mybir as mybir
import concourse.tile as tile
from concourse.bass_utils import run_bass_kernel_spmd

BF16 = mybir.dt.bfloat16
F32 = mybir.dt.float32
AF = mybir.ActivationFunctionType

N_CORES = 8
B, S, D = 16, 2048, 1024
NB = B // N_CORES          # batches per core
KC = D // 128              # 8 chunks of 128 along d
ST = S // 128              # 16 tiles of 128 along s
NBLK = S // 512            # 4 blocks of 512 along s
SCALE = 1.0 / np.sqrt(np.float32(D))  # 1/32


def _split_waits(nc, limit=1):
    """Walrus in this container allows at most one sync wait per instruction:
    hoist excess waits onto NoOp carriers inserted just before."""
    n_new = 0
    for f in nc.m.functions:
        for bb in f.blocks:
            new_insts = []
            for inst in bb.instructions:
                si = inst.sync_info
                waits = list(si.on_wait) if si and si.on_wait else []
                if len(waits) > limit:
                    excess, keep = waits[:-limit], waits[-limit:]
                    for i in range(0, len(excess), limit):
                        chunk = excess[i:i + limit]
                        nop = mybir.InstNoOp(
                            name=f"{inst.name}-ws-{n_new}",
                            ins=[], outs=[],
                            sync_info=mybir.SyncInfo(on_wait=chunk, on_update=[]),
                        )
                        nop.engine = inst.engine
                        new_insts.append(nop)
                        n_new += 1
                    si.on_wait = keep
                new_insts.append(inst)
            bb.instructions[:] = new_insts
    return n_new



def _strip_dead_pe_updates(nc):
    """Drop PE sem increments nobody waits on (Tile emits one per matmul;
    only group-stop indices are ever waited). Renumber wait thresholds by
    rank among kept updates — release timing is identical, PE saves ~26ns
    per dropped serialized EVT_SEM write. Straight-line programs only."""
    pe = mybir.EngineType.PE
    insts = [i for f in nc.m.functions for bb in f.blocks for i in bb.instructions]
    upd_by_sem, wait_by_sem, bad = {}, {}, set()
    for inst in insts:
        si = inst.sync_info
        if not si:
            continue
        for u in (si.on_update or []):
            if u.sync_type != "semaphore":
                continue
            if inst.engine != pe or u.update_mode != "sem-inc" or u.update_value != 1:
                bad.add(u.id)
            upd_by_sem.setdefault(u.id, []).append((inst, u))
        for w in (si.on_wait or []):
            if w.sync_type != "semaphore":
                continue
            if w.wait_mode != "sem-ge-imm" or w.wait_reg is not None:
                bad.add(w.id)
            wait_by_sem.setdefault(w.id, []).append(w)
    n_drop = 0
    for sem_id, ups in upd_by_sem.items():
        if sem_id in bad or sem_id not in wait_by_sem or len(ups) < 16:
            continue
        waited = sorted({w.wait_value for w in wait_by_sem[sem_id]})
        if not waited or waited[-1] > len(ups) or waited[0] < 1:
            continue
        keep = set(waited)
        rank = {t: k + 1 for k, t in enumerate(waited)}
        for idx, (inst, u) in enumerate(ups, start=1):
            if idx not in keep:
                inst.sync_info.on_update = [
                    x for x in inst.sync_info.on_update if x is not u
                ]
                n_drop += 1
        for w in wait_by_sem[sem_id]:
            w.wait_value = rank[w.wait_value]
    return n_drop


def build_program(reps=1):
    """reps>1 wraps the whole computation in a hardware For_i loop — used
    only for timing (slope over reps isolates on-silicon exec time from
    per-call NEFF load overhead)."""
    nc = bass.Bass()

    qT_d = nc.declare_dram_parameter("qT", [NB, D, S], BF16, isOutput=False)
    kT_d = nc.declare_dram_parameter("kT", [NB, D, S], BF16, isOutput=False)
    vT_d = nc.declare_dram_parameter("vT", [NB, D, S], BF16, isOutput=False)
    Wq_d = nc.declare_dram_parameter("Wq", [D, D], BF16, isOutput=False)
    Wk_d = nc.declare_dram_parameter("Wk", [D, D], BF16, isOutput=False)
    Wv_d = nc.declare_dram_parameter("Wv", [D, D], BF16, isOutput=False)
    Wo_d = nc.declare_dram_parameter("Wo", [D, D], BF16, isOutput=False)
    # bq pre-scaled by 1/32 and reshaped [128, KC] host-side; bk likewise unscaled
    bq_d = nc.declare_dram_parameter("bq", [128, KC], F32, isOutput=False)
    bk_d = nc.declare_dram_parameter("bk", [128, KC], F32, isOutput=False)
    bv_d = nc.declare_dram_parameter("bv", [D], BF16, isOutput=False)
    bo_d = nc.declare_dram_parameter("bo", [D], BF16, isOutput=False)
    out_d = nc.declare_dram_parameter("out", [NB, S, D], F32, isOutput=True)

    from contextlib import ExitStack
    with tile.TileContext(nc) as tc:
        with ExitStack() as _stk:
            _p = lambda **kw: _stk.enter_context(tc.tile_pool(**kw))
            wqopool = _p(name="wqo", bufs=8)
            wkvpool = _p(name="wkv", bufs=9)
            inpool = _p(name="inp", bufs=16)
            kpool = _p(name="keyT", bufs=8)
            vpool = _p(name="value", bufs=1)
            qpool = _p(name="queryT", bufs=12)
            epool = _p(name="expT", bufs=2)
            upool = _p(name="UT", bufs=2)
            opool = _p(name="outb", bufs=2)
            sumpool = _p(name="sums", bufs=2)
            rpool = _p(name="rpool", bufs=2)
            cpool = _p(name="const", bufs=1)
            pspool = _p(name="ps", bufs=5, space="PSUM")
            ps1pool = _p(name="ps1", bufs=1, space="PSUM")
            psrpool = _p(name="psr", bufs=2, space="PSUM")
            # constants
            ones = cpool.tile([128, 1], BF16, tag="ones")
            nc.vector.memset(ones[:], 1.0)
            ident = cpool.tile([1, 1], F32, tag="ident")
            nc.vector.memset(ident[:], 1.0)
            bq_sb = cpool.tile([128, KC], F32, tag="bq")
            nc.sync.dma_start(out=bq_sb[:], in_=bq_d[:])
            bk_sb = cpool.tile([128, KC], F32, tag="bk")
            nc.sync.dma_start(out=bk_sb[:], in_=bk_d[:])
            bv_sb = cpool.tile([128, D], BF16, tag="bv")
            ap = bv_d[:]
            nc.sync.dma_start(
                out=bv_sb[:],
                in_=bass.AP(tensor=ap.tensor, offset=ap.offset, ap=[[0, 128]] + ap.ap),
            )
            bo_sb = cpool.tile([128, D], BF16, tag="bo")
            ap = bo_d[:]
            nc.sync.dma_start(
                out=bo_sb[:],
                in_=bass.AP(tensor=ap.tensor, offset=ap.offset, ap=[[0, 128]] + ap.ap),
            )

            def load_w(w_d, pool, tag):
                tiles = []
                for i in range(KC):
                    t = pool.tile([128, D], BF16, tag=tag, name=f"{tag}{i}")
                    nc.sync.dma_start(out=t[:], in_=w_d[i * 128:(i + 1) * 128, :])
                    tiles.append(t)
                return tiles

            # Wq and Wo stay resident for the whole kernel
            Wq_t = load_w(Wq_d, wqopool, "wq")
            Wo_t = load_w(Wo_d, wqopool, "wo")

            import contextlib
            loop_ctx = tc.For_i(0, reps, 1) if reps > 1 else contextlib.nullcontext()
            with loop_ctx:
              for b in range(NB):
                  # ---------------- keyT[d, s] = Wk.T @ kT (+bk) ----------------
                  Wk_t = load_w(Wk_d, wkvpool, "wkv")
                  keyT = [kpool.tile([128, S], BF16, tag="keyT", name=f"keyT{i}") for i in range(KC)]
                  for s in range(NBLK):
                      kin = []
                      for i in range(KC):
                          t = inpool.tile([128, 512], BF16, tag="inp", name=f"in{i}")
                          nc.sync.dma_start(
                              out=t[:],
                              in_=kT_d[b, i * 128:(i + 1) * 128, s * 512:(s + 1) * 512],
                          )
                          kin.append(t)
                      for do in range(KC):
                          psum = pspool.tile([128, 512], F32, tag="ps")
                          for i in range(KC):
                              nc.tensor.matmul(
                                  psum[:], Wk_t[i][:, do * 128:(do + 1) * 128], kin[i][:],
                                  start=(i == 0), stop=(i == KC - 1),
                              )
                          nc.vector.tensor_scalar_add(
                              keyT[do][:, s * 512:(s + 1) * 512], psum[:],
                              bk_sb[:, do:do + 1],
                          )

                  # ---------------- value[s, d] = vT.T @ Wv (+bv) ----------------
                  Wv_t = load_w(Wv_d, wkvpool, "wkv")
                  val = vpool.tile([128, ST, D], BF16, tag="value")
                  for s in range(NBLK):
                      vin = []
                      for i in range(KC):
                          t = inpool.tile([128, 512], BF16, tag="inp", name=f"in{i}")
                          nc.sync.dma_start(
                              out=t[:],
                              in_=vT_d[b, i * 128:(i + 1) * 128, s * 512:(s + 1) * 512],
                          )
                          vin.append(t)
                      for tt in range(4):
                          t16 = s * 4 + tt
                          for n in range(2):
                              psum = pspool.tile([128, 512], F32, tag="ps")
                              for i in range(KC):
                                  nc.tensor.matmul(
                                      psum[:],
                                      vin[i][:, tt * 128:(tt + 1) * 128],
                                      Wv_t[i][:, n * 512:(n + 1) * 512],
                                      start=(i == 0), stop=(i == KC - 1),
                                  )
                              nc.vector.tensor_add(
                                  val[:, t16, n * 512:(n + 1) * 512], psum[:],
                                  bv_sb[:, n * 512:(n + 1) * 512],
                              )

                  # ---------------- per 512-wide sq block ----------------
                  for blk in range(NBLK):
                      # queryT block [d, 512] = Wq.T @ qT_blk, scaled 1/32 (+bq/32)
                      qin = []
                      for i in range(KC):
                          t = inpool.tile([128, 512], BF16, tag="inp", name=f"in{i}")
                          nc.sync.dma_start(
                              out=t[:],
                              in_=qT_d[b, i * 128:(i + 1) * 128, blk * 512:(blk + 1) * 512],
                          )
                          qin.append(t)
                      qry = []
                      for do in range(KC):
                          psum = pspool.tile([128, 512], F32, tag="ps")
                          for i in range(KC):
                              nc.tensor.matmul(
                                  psum[:], Wq_t[i][:, do * 128:(do + 1) * 128], qin[i][:],
                                  start=(i == 0), stop=(i == KC - 1),
                              )
                          qt = qpool.tile([128, 512], BF16, tag="queryT", name=f"qry{do}")
                          nc.vector.tensor_scalar(
                              out=qt[:], in0=psum[:], scalar1=float(SCALE),
                              scalar2=bq_sb[:, do:do + 1],
                              op0=mybir.AluOpType.mult, op1=mybir.AluOpType.add,
                          )
                          qry.append(qt)

                      # scoresT -> expT
                      exp_blk = epool.tile([128, ST, 512], BF16, tag="expT")
                      for t16 in range(ST):
                          psum = pspool.tile([128, 512], F32, tag="ps")
                          for i in range(KC):
                              nc.tensor.matmul(
                                  psum[:],
                                  keyT[i][:, t16 * 128:(t16 + 1) * 128],
                                  qry[i][:],
                                  start=(i == 0), stop=(i == KC - 1),
                              )
                          nc.scalar.activation(exp_blk[:, t16, :], psum[:], AF.Exp)

                      # column sums over all sk (partition dim) via ones-matmul
                      sums_ps = ps1pool.tile([1, 512], F32, tag="ps1")
                      for t16 in range(ST):
                          nc.tensor.matmul(
                              sums_ps[:], ones[:], exp_blk[:, t16, :],
                              start=(t16 == 0), stop=(t16 == ST - 1),
                          )
                      sums_sb = sumpool.tile([1, 512], F32, tag="sums")
                      nc.vector.tensor_copy(sums_sb[:], sums_ps[:])

                      # r = 1/sums as per-partition scalars, via [1,128] PE
                      # transpose; emitted before UT so its PE<->DVE chain is
                      # hidden under the UT matmul stream
                      r_sb = rpool.tile([128, 4], F32, tag="r")
                      for m in range(4):
                          pr = psrpool.tile([128, 1], F32, tag="psr")
                          nc.tensor.transpose(
                              pr[:], sums_sb[0:1, m * 128:(m + 1) * 128], ident[:]
                          )
                          nc.vector.reciprocal(r_sb[:, m:m + 1], pr[:])

                      # UT block [d, 512] = value.T @ expT
                      ut = upool.tile([128, KC, 512], BF16, tag="UT")
                      for j in range(KC):
                          psum = pspool.tile([128, 512], F32, tag="ps")
                          for t16 in range(ST):
                              nc.tensor.matmul(
                                  psum[:],
                                  val[:, t16, j * 128:(j + 1) * 128],
                                  exp_blk[:, t16, :],
                                  start=(t16 == 0), stop=(t16 == ST - 1),
                              )
                          nc.vector.tensor_copy(ut[:, j, :], psum[:])

                      # final block: out[sq, d] = (UT.T @ Wo) * r + bo
                      for m in range(4):
                          ob = opool.tile([128, D], F32, tag="outb")
                          for n in range(2):
                              psum = pspool.tile([128, 512], F32, tag="ps")
                              for j in range(KC):
                                  nc.tensor.matmul(
                                      psum[:],
                                      ut[:, j, m * 128:(m + 1) * 128],
                                      Wo_t[j][:, n * 512:(n + 1) * 512],
                                      start=(j == 0), stop=(j == KC - 1),
                                  )
                              nc.vector.tensor_scalar_mul(
                                  ob[:, n * 512:(n + 1) * 512], psum[:], r_sb[:, m:m + 1]
                              )
                              nc.vector.tensor_add(
                                  ob[:, n * 512:(n + 1) * 512],
                                  ob[:, n * 512:(n + 1) * 512],
                                  bo_sb[:, n * 512:(n + 1) * 512],
                              )
                          sq = blk * 512 + m * 128
                          nc.sync.dma_start(out=out_d[b, sq:sq + 128, :], in_=ob[:])

    if reps == 1:
        _strip_dead_pe_updates(nc)
    _split_waits(nc)
    return nc


_PROGRAM = None


def _get_program():
    global _PROGRAM
    if _PROGRAM is None:
        _PROGRAM = build_program()
    return _PROGRAM


def prepare_in_maps(q, k, v, Wq, bq, Wk, bk, Wv, bv, Wo, bo):
    bf = ml_dtypes.bfloat16
    f32 = np.float32

    def t_bf16(x):  # [B,S,D] f32 -> [B,D,S] bf16 contiguous
        return np.ascontiguousarray(
            np.asarray(x, f32).astype(bf).transpose(0, 2, 1)
        )

    qT = t_bf16(q)
    kT = t_bf16(k)
    vT = t_bf16(v)
    Wq_b = np.asarray(Wq, f32).astype(bf)
    Wk_b = np.asarray(Wk, f32).astype(bf)
    Wv_b = np.asarray(Wv, f32).astype(bf)
    Wo_b = np.asarray(Wo, f32).astype(bf)
    bq2 = np.ascontiguousarray(
        (np.asarray(bq, f32) * np.float32(SCALE)).reshape(KC, 128).T
    )
    bk2 = np.ascontiguousarray(np.asarray(bk, f32).reshape(KC, 128).T)
    bv1 = np.ascontiguousarray(np.asarray(bv, f32)).astype(bf)
    bo1 = np.ascontiguousarray(np.asarray(bo, f32)).astype(bf)

    in_maps = []
    for c in range(N_CORES):
        sl = slice(c * NB, (c + 1) * NB)
        in_maps.append({
            "qT": qT[sl], "kT": kT[sl], "vT": vT[sl],
            "Wq": Wq_b, "Wk": Wk_b, "Wv": Wv_b, "Wo": Wo_b,
            "bq": bq2, "bk": bk2, "bv": bv1, "bo": bo1,
        })
    return in_maps


def kernel(q, k, v, Wq, bq, Wk, bk, Wv, bv, Wo, bo):
    nc = _get_program()
    in_maps = prepare_in_maps(q, k, v, Wq, bq, Wk, bk, Wv, bv, Wo, bo)
    res = run_bass_kernel_spmd(nc, in_maps, core_ids=list(range(N_CORES)))
    out = np.concatenate([res.results[c]["out"] for c in range(N_CORES)], axis=0)
    return out.astype(np.float32)



# revision 2
# speedup vs baseline: 1.0434x; 1.0434x over previous
"""CrossAttention (single-head) Trainium2 kernel, 8-core data-parallel.

Full inputs in, full output out. Internally: batch 16 is sharded 2-per-core
across 8 NeuronCores; each core runs the whole attention layer for its two
batches in bf16 (f32 PSUM accumulation), with activations kept in transposed
[d, s] layout so every matmul contracts over the partition dim without any
on-chip transposes of large tensors.

v2 perf changes vs baseline:
- All multi-tile loads (weights, per-block activations) are single-trigger
  3D-AP DMAs: fewer Sync-engine trigger slots, less HWDGE FIFO serialization.
- DMA emission order puts batch-0 K-projection inputs (kin, Wk halves, bk)
  first so the first matmul starts ~9us in instead of ~31us.
- Softmax denominators: the 16 exp tiles are pair-folded on the Vector
  engine down to 2 accumulators, so the partition-sum ones-matmul chain is
  2 matmuls per block instead of 16 (saves ~24us of PE time).
"""

import sys

sys.path.insert(0, "/opt/trn_rl_repo")

import numpy as np
import ml_dtypes

import concourse.bass as bass
import concourse.mybir as mybir
import concourse.tile as tile
from concourse.bass_utils import run_bass_kernel_spmd

BF16 = mybir.dt.bfloat16
F32 = mybir.dt.float32
AF = mybir.ActivationFunctionType

N_CORES = 8
B, S, D = 16, 2048, 1024
NB = B // N_CORES          # batches per core
KC = D // 128              # 8 chunks of 128 along d
ST = S // 128              # 16 tiles of 128 along s
NBLK = S // 512            # 4 blocks of 512 along s
SCALE = 1.0 / np.sqrt(np.float32(D))  # 1/32


def _split_waits(nc, limit=1):
    """Walrus in this container allows at most one sync wait per instruction:
    hoist excess waits onto NoOp carriers inserted just before."""
    n_new = 0
    for f in nc.m.functions:
        for bb in f.blocks:
            new_insts = []
            for inst in bb.instructions:
                si = inst.sync_info
                waits = list(si.on_wait) if si and si.on_wait else []
                if len(waits) > limit:
                    excess, keep = waits[:-limit], waits[-limit:]
                    for i in range(0, len(excess), limit):
                        chunk = excess[i:i + limit]
                        nop = mybir.InstNoOp(
                            name=f"{inst.name}-ws-{n_new}",
                            ins=[], outs=[],
                            sync_info=mybir.SyncInfo(on_wait=chunk, on_update=[]),
                        )
                        nop.engine = inst.engine
                        new_insts.append(nop)
                        n_new += 1
                    si.on_wait = keep
                new_insts.append(inst)
            bb.instructions[:] = new_insts
    return n_new


def _strip_dead_pe_updates(nc):
    """Drop PE sem increments nobody waits on (Tile emits one per matmul;
    only group-stop indices are ever waited). Renumber wait thresholds by
    rank among kept updates — release timing is identical, PE saves ~26ns
    per dropped serialized EVT_SEM write. Straight-line programs only."""
    pe = mybir.EngineType.PE
    insts = [i for f in nc.m.functions for bb in f.blocks for i in bb.instructions]
    upd_by_sem, wait_by_sem, bad = {}, {}, set()
    for inst in insts:
        si = inst.sync_info
        if not si:
            continue
        for u in (si.on_update or []):
            if u.sync_type != "semaphore":
                continue
            if inst.engine != pe or u.update_mode != "sem-inc" or u.update_value != 1:
                bad.add(u.id)
            upd_by_sem.setdefault(u.id, []).append((inst, u))
        for w in (si.on_wait or []):
            if w.sync_type != "semaphore":
                continue
            if w.wait_mode != "sem-ge-imm" or w.wait_reg is not None:
                bad.add(w.id)
            wait_by_sem.setdefault(w.id, []).append(w)
    n_drop = 0
    for sem_id, ups in upd_by_sem.items():
        if sem_id in bad or sem_id not in wait_by_sem or len(ups) < 16:
            continue
        waited = sorted({w.wait_value for w in wait_by_sem[sem_id]})
        if not waited or waited[-1] > len(ups) or waited[0] < 1:
            continue
        keep = set(waited)
        rank = {t: k + 1 for k, t in enumerate(waited)}
        for idx, (inst, u) in enumerate(ups, start=1):
            if idx not in keep:
                inst.sync_info.on_update = [
                    x for x in inst.sync_info.on_update if x is not u
                ]
                n_drop += 1
        for w in wait_by_sem[sem_id]:
            w.wait_value = rank[w.wait_value]
    return n_drop


def build_program(reps=1):
    nc = bass.Bass()

    qT_d = nc.declare_dram_parameter("qT", [NB, D, S], BF16, isOutput=False)
    kT_d = nc.declare_dram_parameter("kT", [NB, D, S], BF16, isOutput=False)
    vT_d = nc.declare_dram_parameter("vT", [NB, D, S], BF16, isOutput=False)
    Wq_d = nc.declare_dram_parameter("Wq", [D, D], BF16, isOutput=False)
    Wk_d = nc.declare_dram_parameter("Wk", [D, D], BF16, isOutput=False)
    Wv_d = nc.declare_dram_parameter("Wv", [D, D], BF16, isOutput=False)
    Wo_d = nc.declare_dram_parameter("Wo", [D, D], BF16, isOutput=False)
    # bq pre-scaled by 1/32 and reshaped [128, KC] host-side; bk likewise unscaled
    bq_d = nc.declare_dram_parameter("bq", [128, KC], F32, isOutput=False)
    bk_d = nc.declare_dram_parameter("bk", [128, KC], F32, isOutput=False)
    bv_d = nc.declare_dram_parameter("bv", [D], BF16, isOutput=False)
    bo_d = nc.declare_dram_parameter("bo", [D], BF16, isOutput=False)
    out_d = nc.declare_dram_parameter("out", [NB, S, D], F32, isOutput=True)

    def w_ap(w_d, col0, ncol):
        """[D, D] weight -> SBUF [128, KC, ncol] chunk-major AP (cols
        col0:col0+ncol of every 128-row chunk) in one DMA."""
        ap = w_d[:]
        return bass.AP(
            tensor=ap.tensor, offset=ap.offset + col0,
            ap=[[D, 128], [128 * D, KC], [1, ncol]],
        )

    def x_ap(x_d, b, s0, ncol):
        """[NB, D, S] activation -> SBUF [128, KC, ncol] chunk-major AP."""
        ap = x_d[:]
        return bass.AP(
            tensor=ap.tensor, offset=ap.offset + b * D * S + s0,
            ap=[[S, 128], [128 * S, KC], [1, ncol]],
        )

    def bcast_ap(v_d):
        ap = v_d[:]
        return bass.AP(tensor=ap.tensor, offset=ap.offset, ap=[[0, 128]] + ap.ap)

    from contextlib import ExitStack
    with tile.TileContext(nc) as tc:
        with ExitStack() as _stk:
            _p = lambda **kw: _stk.enter_context(tc.tile_pool(**kw))
            wqopool = _p(name="wqo", bufs=1)
            wkvpool = _p(name="wkv", bufs=2)
            inpool = _p(name="inp", bufs=3)
            kpool = _p(name="keyT", bufs=8)
            vpool = _p(name="value", bufs=1)
            qpool = _p(name="queryT", bufs=1)
            epool = _p(name="expT", bufs=1)
            fpool = _p(name="fold", bufs=2)
            upool = _p(name="UT", bufs=1)
            opool = _p(name="outb", bufs=2)
            sumpool = _p(name="sums", bufs=2)
            rpool = _p(name="rpool", bufs=2)
            cpool = _p(name="const", bufs=1)
            pspool = _p(name="ps", bufs=5, space="PSUM")
            ps1pool = _p(name="ps1", bufs=1, space="PSUM")
            psrpool = _p(name="psr", bufs=2, space="PSUM")

            # constants (cheap memsets; no DMA)
            ones = cpool.tile([128, 1], BF16, tag="ones")
            nc.vector.memset(ones[:], 1.0)
            ident = cpool.tile([1, 1], F32, tag="ident")
            nc.vector.memset(ident[:], 1.0)

            # ---- batch-0 critical-path DMAs first: kin(sblk0), Wk, bk ----
            kin0 = inpool.tile([128, KC, 512], BF16, tag="inp", name="kin0")
            nc.sync.dma_start(out=kin0[:], in_=x_ap(kT_d, 0, 0, 512))
            wk_b = [None] * NB
            wv_b = [None] * NB
            wk_b[0] = wkvpool.tile([128, KC, D], BF16, tag="wkv", name="wk0")
            # two half-column triggers so the first 4 output chains can
            # start after ~half the weight transfer
            nc.sync.dma_start(out=wk_b[0][:, :, 0:512], in_=w_ap(Wk_d, 0, 512))
            nc.sync.dma_start(out=wk_b[0][:, :, 512:D], in_=w_ap(Wk_d, 512, 512))
            bk_sb = cpool.tile([128, KC], F32, tag="bk")
            nc.sync.dma_start(out=bk_sb[:], in_=bk_d[:])

            # deferred-load tiles (DMAs emitted mid-stream below)
            bq_sb = cpool.tile([128, KC], F32, tag="bq")
            bv_sb = cpool.tile([128, D], BF16, tag="bv")
            bo_sb = cpool.tile([128, D], BF16, tag="bo")
            wq = wqopool.tile([128, KC, D], BF16, tag="wq")
            wo = wqopool.tile([128, KC, D], BF16, tag="wo")

            import contextlib
            loop_ctx = tc.For_i(0, reps, 1) if reps > 1 else contextlib.nullcontext()
            with loop_ctx:
              for b in range(NB):
                  if b > 0:
                      wk_b[b] = wkvpool.tile([128, KC, D], BF16, tag="wkv",
                                             name=f"wk{b}")
                      nc.sync.dma_start(out=wk_b[b][:], in_=w_ap(Wk_d, 0, D))
                  wk = wk_b[b]

                  # ---------------- keyT[d, s] = Wk.T @ kT (+bk) ----------------
                  keyT = [kpool.tile([128, S], BF16, tag="keyT", name=f"keyT{i}")
                          for i in range(KC)]
                  for s in range(NBLK):
                      if b == 0 and s == 0:
                          kin = kin0
                      else:
                          kin = inpool.tile([128, KC, 512], BF16, tag="inp",
                                            name=f"kin{s}")
                          nc.sync.dma_start(out=kin[:], in_=x_ap(kT_d, b, s * 512, 512))
                      for do in range(KC):
                          psum = pspool.tile([128, 512], F32, tag="ps")
                          for i in range(KC):
                              nc.tensor.matmul(
                                  psum[:], wk[:, i, do * 128:(do + 1) * 128],
                                  kin[:, i, :],
                                  start=(i == 0), stop=(i == KC - 1),
                              )
                          nc.vector.tensor_scalar_add(
                              keyT[do][:, s * 512:(s + 1) * 512], psum[:],
                              bk_sb[:, do:do + 1],
                          )
                      if b == 0 and s == 0:
                          # Wv + bv arrive during remaining K-proj compute
                          wv_b[0] = wkvpool.tile([128, KC, D], BF16, tag="wkv",
                                                 name="wv0")
                          nc.sync.dma_start(out=wv_b[0][:], in_=w_ap(Wv_d, 0, D))
                          nc.sync.dma_start(out=bv_sb[:], in_=bcast_ap(bv_d))

                  if b > 0:
                      wv_b[b] = wkvpool.tile([128, KC, D], BF16, tag="wkv",
                                             name=f"wv{b}")
                      nc.sync.dma_start(out=wv_b[b][:], in_=w_ap(Wv_d, 0, D))
                  wv = wv_b[b]

                  # ---------------- value[s, d] = vT.T @ Wv (+bv) ----------------
                  val = vpool.tile([128, ST, D], BF16, tag="value")
                  for s in range(NBLK):
                      vin = inpool.tile([128, KC, 512], BF16, tag="inp",
                                        name=f"vin{s}")
                      nc.sync.dma_start(out=vin[:], in_=x_ap(vT_d, b, s * 512, 512))
                      for tt in range(4):
                          t16 = s * 4 + tt
                          for n in range(2):
                              psum = pspool.tile([128, 512], F32, tag="ps")
                              for i in range(KC):
                                  nc.tensor.matmul(
                                      psum[:],
                                      vin[:, i, tt * 128:(tt + 1) * 128],
                                      wv[:, i, n * 512:(n + 1) * 512],
                                      start=(i == 0), stop=(i == KC - 1),
                                  )
                              nc.vector.tensor_add(
                                  val[:, t16, n * 512:(n + 1) * 512], psum[:],
                                  bv_sb[:, n * 512:(n + 1) * 512],
                              )
                      if b == 0 and s == 0:
                          # Wq/bq land before the first qry block; Wo/bo are
                          # only read in the final phase, much later
                          nc.sync.dma_start(out=bq_sb[:], in_=bq_d[:])
                          nc.sync.dma_start(out=wq[:], in_=w_ap(Wq_d, 0, D))
                      if b == 0 and s == 2:
                          nc.sync.dma_start(out=wo[:], in_=w_ap(Wo_d, 0, D))
                          nc.sync.dma_start(out=bo_sb[:], in_=bcast_ap(bo_d))

                  # ---------------- per 512-wide sq block ----------------
                  for blk in range(NBLK):
                      # queryT block [d, 512] = Wq.T @ qT_blk, scaled 1/32 (+bq/32)
                      qin = inpool.tile([128, KC, 512], BF16, tag="inp",
                                        name=f"qin{blk}")
                      nc.sync.dma_start(out=qin[:], in_=x_ap(qT_d, b, blk * 512, 512))
                      qry = qpool.tile([128, KC, 512], BF16, tag="queryT")
                      for do in range(KC):
                          psum = pspool.tile([128, 512], F32, tag="ps")
                          for i in range(KC):
                              nc.tensor.matmul(
                                  psum[:], wq[:, i, do * 128:(do + 1) * 128],
                                  qin[:, i, :],
                                  start=(i == 0), stop=(i == KC - 1),
                              )
                          nc.vector.tensor_scalar(
                              out=qry[:, do, :], in0=psum[:], scalar1=float(SCALE),
                              scalar2=bq_sb[:, do:do + 1],
                              op0=mybir.AluOpType.mult, op1=mybir.AluOpType.add,
                          )

                      # scoresT -> expT, with pairwise DVE fold of exp tiles
                      # into 2 accumulators for the partition-sum
                      exp_blk = epool.tile([128, ST, 512], BF16, tag="expT")
                      facc = [
                          fpool.tile([128, 512], BF16, tag="fold", name="facc0"),
                          fpool.tile([128, 512], BF16, tag="fold", name="facc1"),
                      ]
                      for t16 in range(ST):
                          psum = pspool.tile([128, 512], F32, tag="ps")
                          for i in range(KC):
                              nc.tensor.matmul(
                                  psum[:],
                                  keyT[i][:, t16 * 128:(t16 + 1) * 128],
                                  qry[:, i, :],
                                  start=(i == 0), stop=(i == KC - 1),
                              )
                          nc.scalar.activation(exp_blk[:, t16, :], psum[:], AF.Exp)
                          half = t16 // 8
                          if t16 % 8 == 1:
                              nc.vector.tensor_add(
                                  facc[half][:], exp_blk[:, t16 - 1, :],
                                  exp_blk[:, t16, :],
                              )
                          elif t16 % 8 > 1:
                              nc.vector.tensor_add(
                                  facc[half][:], facc[half][:],
                                  exp_blk[:, t16, :],
                              )

                      # column sums over all sk (partition dim): 2 ones-matmuls
                      sums_ps = ps1pool.tile([1, 512], F32, tag="ps1")
                      nc.tensor.matmul(sums_ps[:], ones[:], facc[0][:],
                                       start=True, stop=False)
                      nc.tensor.matmul(sums_ps[:], ones[:], facc[1][:],
                                       start=False, stop=True)
                      sums_sb = sumpool.tile([1, 512], F32, tag="sums")
                      nc.vector.tensor_copy(sums_sb[:], sums_ps[:])

                      # r = 1/sums as per-partition scalars, via [1,128] PE
                      # transpose; emitted before UT so its PE<->DVE chain is
                      # hidden under the UT matmul stream
                      r_sb = rpool.tile([128, 4], F32, tag="r")
                      for m in range(4):
                          pr = psrpool.tile([128, 1], F32, tag="psr")
                          nc.tensor.transpose(
                              pr[:], sums_sb[0:1, m * 128:(m + 1) * 128], ident[:]
                          )
                          nc.vector.reciprocal(r_sb[:, m:m + 1], pr[:])

                      # UT block [d, 512] = value.T @ expT
                      ut = upool.tile([128, KC, 512], BF16, tag="UT")
                      for j in range(KC):
                          psum = pspool.tile([128, 512], F32, tag="ps")
                          for t16 in range(ST):
                              nc.tensor.matmul(
                                  psum[:],
                                  val[:, t16, j * 128:(j + 1) * 128],
                                  exp_blk[:, t16, :],
                                  start=(t16 == 0), stop=(t16 == ST - 1),
                              )
                          nc.vector.tensor_copy(ut[:, j, :], psum[:])

                      # final block: out[sq, d] = (UT.T @ Wo) * r + bo
                      for m in range(4):
                          ob = opool.tile([128, D], F32, tag="outb")
                          for n in range(2):
                              psum = pspool.tile([128, 512], F32, tag="ps")
                              for j in range(KC):
                                  nc.tensor.matmul(
                                      psum[:],
                                      ut[:, j, m * 128:(m + 1) * 128],
                                      wo[:, j, n * 512:(n + 1) * 512],
                                      start=(j == 0), stop=(j == KC - 1),
                                  )
                              nc.vector.tensor_scalar_mul(
                                  ob[:, n * 512:(n + 1) * 512], psum[:], r_sb[:, m:m + 1]
                              )
                              nc.vector.tensor_add(
                                  ob[:, n * 512:(n + 1) * 512],
                                  ob[:, n * 512:(n + 1) * 512],
                                  bo_sb[:, n * 512:(n + 1) * 512],
                              )
                          sq = blk * 512 + m * 128
                          nc.sync.dma_start(out=out_d[b, sq:sq + 128, :], in_=ob[:])

    if reps == 1:
        _strip_dead_pe_updates(nc)
    _split_waits(nc)
    return nc


_PROGRAM = None


def _get_program():
    global _PROGRAM
    if _PROGRAM is None:
        _PROGRAM = build_program()
    return _PROGRAM


def prepare_in_maps(q, k, v, Wq, bq, Wk, bk, Wv, bv, Wo, bo):
    bf = ml_dtypes.bfloat16
    f32 = np.float32

    def t_bf16(x):  # [B,S,D] f32 -> [B,D,S] bf16 contiguous
        return np.ascontiguousarray(
            np.asarray(x, f32).astype(bf).transpose(0, 2, 1)
        )

    qT = t_bf16(q)
    kT = t_bf16(k)
    vT = t_bf16(v)
    Wq_b = np.asarray(Wq, f32).astype(bf)
    Wk_b = np.asarray(Wk, f32).astype(bf)
    Wv_b = np.asarray(Wv, f32).astype(bf)
    Wo_b = np.asarray(Wo, f32).astype(bf)
    bq2 = np.ascontiguousarray(
        (np.asarray(bq, f32) * np.float32(SCALE)).reshape(KC, 128).T
    )
    bk2 = np.ascontiguousarray(np.asarray(bk, f32).reshape(KC, 128).T)
    bv1 = np.ascontiguousarray(np.asarray(bv, f32)).astype(bf)
    bo1 = np.ascontiguousarray(np.asarray(bo, f32)).astype(bf)

    in_maps = []
    for c in range(N_CORES):
        sl = slice(c * NB, (c + 1) * NB)
        in_maps.append({
            "qT": qT[sl], "kT": kT[sl], "vT": vT[sl],
            "Wq": Wq_b, "Wk": Wk_b, "Wv": Wv_b, "Wo": Wo_b,
            "bq": bq2, "bk": bk2, "bv": bv1, "bo": bo1,
        })
    return in_maps


def kernel(q, k, v, Wq, bq, Wk, bk, Wv, bv, Wo, bo):
    nc = _get_program()
    in_maps = prepare_in_maps(q, k, v, Wq, bq, Wk, bk, Wv, bv, Wo, bo)
    res = run_bass_kernel_spmd(nc, in_maps, core_ids=list(range(N_CORES)))
    out = np.concatenate([res.results[c]["out"] for c in range(N_CORES)], axis=0)
    return out.astype(np.float32)


# revision 8
# speedup vs baseline: 1.0461x; 1.0026x over previous
"""CrossAttention (single-head) Trainium2 kernel, 8-core data-parallel.

Full inputs in, full output out. Internally: batch 16 is sharded 2-per-core
across 8 NeuronCores; each core runs the whole attention layer for its two
batches in bf16 (f32 PSUM accumulation), with activations kept in transposed
[d, s] layout so every matmul contracts over the partition dim without any
on-chip transposes of large tensors.

v2 perf changes vs baseline:
- All multi-tile loads (weights, per-block activations) are single-trigger
  3D-AP DMAs: fewer Sync-engine trigger slots, less HWDGE FIFO serialization.
- DMA emission order puts batch-0 K-projection inputs (kin, Wk halves, bk)
  first so the first matmul starts ~9us in instead of ~31us.
- Softmax denominators: the 16 exp tiles are pair-folded on the Vector
  engine down to 2 accumulators, so the partition-sum ones-matmul chain is
  2 matmuls per block instead of 16 (saves ~24us of PE time).
"""

import sys

sys.path.insert(0, "/opt/trn_rl_repo")

import numpy as np
import ml_dtypes

import concourse.bass as bass
import concourse.mybir as mybir
import concourse.tile as tile
from concourse.bass_utils import run_bass_kernel_spmd

BF16 = mybir.dt.bfloat16
F32 = mybir.dt.float32
AF = mybir.ActivationFunctionType

N_CORES = 8
B, S, D = 16, 2048, 1024
NB = B // N_CORES          # batches per core
KC = D // 128              # 8 chunks of 128 along d
ST = S // 128              # 16 tiles of 128 along s
NBLK = S // 512            # 4 blocks of 512 along s
SCALE = 1.0 / np.sqrt(np.float32(D))  # 1/32


def _split_waits(nc, limit=1):
    """Walrus in this container allows at most one sync wait per instruction:
    hoist excess waits onto NoOp carriers inserted just before."""
    n_new = 0
    for f in nc.m.functions:
        for bb in f.blocks:
            new_insts = []
            for inst in bb.instructions:
                si = inst.sync_info
                waits = list(si.on_wait) if si and si.on_wait else []
                if len(waits) > limit:
                    excess, keep = waits[:-limit], waits[-limit:]
                    for i in range(0, len(excess), limit):
                        chunk = excess[i:i + limit]
                        nop = mybir.InstNoOp(
                            name=f"{inst.name}-ws-{n_new}",
                            ins=[], outs=[],
                            sync_info=mybir.SyncInfo(on_wait=chunk, on_update=[]),
                        )
                        nop.engine = inst.engine
                        new_insts.append(nop)
                        n_new += 1
                    si.on_wait = keep
                new_insts.append(inst)
            bb.instructions[:] = new_insts
    return n_new


def _strip_dead_pe_updates(nc):
    """Drop PE sem increments nobody waits on (Tile emits one per matmul;
    only group-stop indices are ever waited). Renumber wait thresholds by
    rank among kept updates — release timing is identical, PE saves ~26ns
    per dropped serialized EVT_SEM write. Straight-line programs only."""
    pe = mybir.EngineType.PE
    insts = [i for f in nc.m.functions for bb in f.blocks for i in bb.instructions]
    upd_by_sem, wait_by_sem, bad = {}, {}, set()
    for inst in insts:
        si = inst.sync_info
        if not si:
            continue
        for u in (si.on_update or []):
            if u.sync_type != "semaphore":
                continue
            if inst.engine != pe or u.update_mode != "sem-inc" or u.update_value != 1:
                bad.add(u.id)
            upd_by_sem.setdefault(u.id, []).append((inst, u))
        for w in (si.on_wait or []):
            if w.sync_type != "semaphore":
                continue
            if w.wait_mode != "sem-ge-imm" or w.wait_reg is not None:
                bad.add(w.id)
            wait_by_sem.setdefault(w.id, []).append(w)
    n_drop = 0
    for sem_id, ups in upd_by_sem.items():
        if sem_id in bad or sem_id not in wait_by_sem or len(ups) < 16:
            continue
        waited = sorted({w.wait_value for w in wait_by_sem[sem_id]})
        if not waited or waited[-1] > len(ups) or waited[0] < 1:
            continue
        keep = set(waited)
        rank = {t: k + 1 for k, t in enumerate(waited)}
        for idx, (inst, u) in enumerate(ups, start=1):
            if idx not in keep:
                inst.sync_info.on_update = [
                    x for x in inst.sync_info.on_update if x is not u
                ]
                n_drop += 1
        for w in wait_by_sem[sem_id]:
            w.wait_value = rank[w.wait_value]
    return n_drop


def build_program(reps=1):
    nc = bass.Bass()

    qT_d = nc.declare_dram_parameter("qT", [NB, D, S], BF16, isOutput=False)
    kT_d = nc.declare_dram_parameter("kT", [NB, D, S], BF16, isOutput=False)
    vT_d = nc.declare_dram_parameter("vT", [NB, D, S], BF16, isOutput=False)
    Wq_d = nc.declare_dram_parameter("Wq", [D, D], BF16, isOutput=False)
    Wk_d = nc.declare_dram_parameter("Wk", [D, D], BF16, isOutput=False)
    Wv_d = nc.declare_dram_parameter("Wv", [D, D], BF16, isOutput=False)
    Wo_d = nc.declare_dram_parameter("Wo", [D, D], BF16, isOutput=False)
    # bq pre-scaled by 1/32 and reshaped [128, KC] host-side; bk likewise unscaled
    bq_d = nc.declare_dram_parameter("bq", [128, KC], F32, isOutput=False)
    bk_d = nc.declare_dram_parameter("bk", [128, KC], F32, isOutput=False)
    bv_d = nc.declare_dram_parameter("bv", [D], BF16, isOutput=False)
    bo_d = nc.declare_dram_parameter("bo", [D], BF16, isOutput=False)
    out_d = nc.declare_dram_parameter("out", [NB, S, D], F32, isOutput=True)

    def w_ap(w_d, col0, ncol):
        """[D, D] weight -> SBUF [128, KC, ncol] chunk-major AP (cols
        col0:col0+ncol of every 128-row chunk) in one DMA."""
        ap = w_d[:]
        return bass.AP(
            tensor=ap.tensor, offset=ap.offset + col0,
            ap=[[D, 128], [128 * D, KC], [1, ncol]],
        )

    def x_ap(x_d, b, s0, ncol):
        """[NB, D, S] activation -> SBUF [128, KC, ncol] chunk-major AP."""
        ap = x_d[:]
        return bass.AP(
            tensor=ap.tensor, offset=ap.offset + b * D * S + s0,
            ap=[[S, 128], [128 * S, KC], [1, ncol]],
        )

    def bcast_ap(v_d):
        ap = v_d[:]
        return bass.AP(tensor=ap.tensor, offset=ap.offset, ap=[[0, 128]] + ap.ap)

    from contextlib import ExitStack
    with tile.TileContext(nc) as tc:
        with ExitStack() as _stk:
            _p = lambda **kw: _stk.enter_context(tc.tile_pool(**kw))
            wqopool = _p(name="wqo", bufs=1)
            wkvpool = _p(name="wkv", bufs=2)
            inpool = _p(name="inp", bufs=3)
            kpool = _p(name="keyT", bufs=8)
            vpool = _p(name="value", bufs=1)
            qpool = _p(name="queryT", bufs=1)
            epool = _p(name="expT", bufs=1)
            fpool = _p(name="fold", bufs=2)
            upool = _p(name="UT", bufs=1)
            opool = _p(name="outb", bufs=2)
            sumpool = _p(name="sums", bufs=2)
            rpool = _p(name="rpool", bufs=2)
            cpool = _p(name="const", bufs=1)
            pspool = _p(name="ps", bufs=5, space="PSUM")
            ps1pool = _p(name="ps1", bufs=1, space="PSUM")
            psrpool = _p(name="psr", bufs=2, space="PSUM")

            # constants (cheap memsets; no DMA)
            ones = cpool.tile([128, 1], BF16, tag="ones")
            nc.vector.memset(ones[:], 1.0)
            ident = cpool.tile([1, 1], F32, tag="ident")
            nc.vector.memset(ident[:], 1.0)

            # ---- batch-0 critical-path DMAs first: kin(sblk0), Wk, bk ----
            # Split so the first chain's first-half accumulation (kin chunks
            # 0-3 x Wk chunks 0-3) can start after ~1MB of transfer; subtile
            # deps gate each matmul on just the DMA covering its region.
            kin0 = inpool.tile([128, KC, 512], BF16, tag="inp", name="kin0")
            wk_b = [None] * NB
            wv_b = [None] * NB
            wk_b[0] = wkvpool.tile([128, KC, D], BF16, tag="wkv", name="wk0")

            def half_x_ap(x_d, b, s0, ch0):
                ap = x_d[:]
                return bass.AP(
                    tensor=ap.tensor,
                    offset=ap.offset + b * D * S + ch0 * 128 * S + s0,
                    ap=[[S, 128], [128 * S, KC // 2], [1, 512]],
                )

            def half_w_ap(w_d, col0, ncol, ch0):
                ap = w_d[:]
                return bass.AP(
                    tensor=ap.tensor, offset=ap.offset + ch0 * 128 * D + col0,
                    ap=[[D, 128], [128 * D, KC // 2], [1, ncol]],
                )

            nc.sync.dma_start(out=kin0[:, 0:4, :], in_=half_x_ap(kT_d, 0, 0, 0))
            nc.sync.dma_start(out=wk_b[0][:, 0:4, 0:512], in_=half_w_ap(Wk_d, 0, 512, 0))
            nc.sync.dma_start(out=kin0[:, 4:8, :], in_=half_x_ap(kT_d, 0, 0, 4))
            nc.sync.dma_start(out=wk_b[0][:, 4:8, 0:512], in_=half_w_ap(Wk_d, 0, 512, 4))
            nc.sync.dma_start(out=wk_b[0][:, :, 512:D], in_=w_ap(Wk_d, 512, 512))
            bk_sb = cpool.tile([128, KC], F32, tag="bk")
            nc.sync.dma_start(out=bk_sb[:], in_=bk_d[:])

            # deferred-load tiles (DMAs emitted mid-stream below)
            bq_sb = cpool.tile([128, KC], F32, tag="bq")
            bv_sb = cpool.tile([128, D], BF16, tag="bv")
            bo_sb = cpool.tile([128, D], BF16, tag="bo")
            wq = wqopool.tile([128, KC, D], BF16, tag="wq")
            wo = wqopool.tile([128, KC, D], BF16, tag="wo")

            import contextlib
            loop_ctx = tc.For_i(0, reps, 1) if reps > 1 else contextlib.nullcontext()
            with loop_ctx:
              for b in range(NB):
                  if b > 0:
                      wk_b[b] = wkvpool.tile([128, KC, D], BF16, tag="wkv",
                                             name=f"wk{b}")
                      nc.sync.dma_start(out=wk_b[b][:], in_=w_ap(Wk_d, 0, D))
                  wk = wk_b[b]

                  # ---------------- keyT[d, s] = Wk.T @ kT (+bk) ----------------
                  keyT = [kpool.tile([128, S], BF16, tag="keyT", name=f"keyT{i}")
                          for i in range(KC)]
                  for s in range(NBLK):
                      if b == 0 and s == 0:
                          kin = kin0
                      else:
                          kin = inpool.tile([128, KC, 512], BF16, tag="inp",
                                            name=f"kin{s}")
                          nc.sync.dma_start(out=kin[:], in_=x_ap(kT_d, b, s * 512, 512))
                      for do in range(KC):
                          psum = pspool.tile([128, 512], F32, tag="ps")
                          for i in range(KC):
                              nc.tensor.matmul(
                                  psum[:], wk[:, i, do * 128:(do + 1) * 128],
                                  kin[:, i, :],
                                  start=(i == 0), stop=(i == KC - 1),
                              )
                          nc.vector.tensor_scalar_add(
                              keyT[do][:, s * 512:(s + 1) * 512], psum[:],
                              bk_sb[:, do:do + 1],
                          )
                      if b == 0 and s == 0:
                          # Wv + bv arrive during remaining K-proj compute
                          wv_b[0] = wkvpool.tile([128, KC, D], BF16, tag="wkv",
                                                 name="wv0")
                          nc.sync.dma_start(out=wv_b[0][:], in_=w_ap(Wv_d, 0, D))
                          nc.sync.dma_start(out=bv_sb[:], in_=bcast_ap(bv_d))

                  if b > 0:
                      wv_b[b] = wkvpool.tile([128, KC, D], BF16, tag="wkv",
                                             name=f"wv{b}")
                      nc.sync.dma_start(out=wv_b[b][:], in_=w_ap(Wv_d, 0, D))
                  wv = wv_b[b]

                  # ---------------- value[s, d] = vT.T @ Wv (+bv) ----------------
                  val = vpool.tile([128, ST, D], BF16, tag="value")
                  for s in range(NBLK):
                      vin = inpool.tile([128, KC, 512], BF16, tag="inp",
                                        name=f"vin{s}")
                      nc.sync.dma_start(out=vin[:], in_=x_ap(vT_d, b, s * 512, 512))
                      for tt in range(4):
                          t16 = s * 4 + tt
                          for n in range(2):
                              psum = pspool.tile([128, 512], F32, tag="ps")
                              for i in range(KC):
                                  nc.tensor.matmul(
                                      psum[:],
                                      vin[:, i, tt * 128:(tt + 1) * 128],
                                      wv[:, i, n * 512:(n + 1) * 512],
                                      start=(i == 0), stop=(i == KC - 1),
                                  )
                              nc.vector.tensor_add(
                                  val[:, t16, n * 512:(n + 1) * 512], psum[:],
                                  bv_sb[:, n * 512:(n + 1) * 512],
                              )
                      if b == 0 and s == 0:
                          # Wq/bq land before the first qry block; Wo/bo are
                          # only read in the final phase, much later
                          nc.sync.dma_start(out=bq_sb[:], in_=bq_d[:])
                          nc.sync.dma_start(out=wq[:], in_=w_ap(Wq_d, 0, D))
                      if b == 0 and s == 2:
                          nc.sync.dma_start(out=wo[:], in_=w_ap(Wo_d, 0, D))
                          nc.sync.dma_start(out=bo_sb[:], in_=bcast_ap(bo_d))

                  # ---------------- per 512-wide sq block ----------------
                  for blk in range(NBLK):
                      # queryT block [d, 512] = Wq.T @ qT_blk, scaled 1/32 (+bq/32)
                      qin = inpool.tile([128, KC, 512], BF16, tag="inp",
                                        name=f"qin{blk}")
                      nc.sync.dma_start(out=qin[:], in_=x_ap(qT_d, b, blk * 512, 512))
                      qry = qpool.tile([128, KC, 512], BF16, tag="queryT")
                      for do in range(KC):
                          psum = pspool.tile([128, 512], F32, tag="ps")
                          for i in range(KC):
                              nc.tensor.matmul(
                                  psum[:], wq[:, i, do * 128:(do + 1) * 128],
                                  qin[:, i, :],
                                  start=(i == 0), stop=(i == KC - 1),
                              )
                          nc.vector.tensor_scalar(
                              out=qry[:, do, :], in0=psum[:], scalar1=float(SCALE),
                              scalar2=bq_sb[:, do:do + 1],
                              op0=mybir.AluOpType.mult, op1=mybir.AluOpType.add,
                          )

                      # scoresT -> expT, with pairwise DVE fold of exp tiles
                      # into 2 accumulators for the partition-sum
                      exp_blk = epool.tile([128, ST, 512], BF16, tag="expT")
                      facc = [
                          fpool.tile([128, 512], BF16, tag="fold", name="facc0"),
                          fpool.tile([128, 512], BF16, tag="fold", name="facc1"),
                      ]
                      for t16 in range(ST):
                          psum = pspool.tile([128, 512], F32, tag="ps")
                          for i in range(KC):
                              nc.tensor.matmul(
                                  psum[:],
                                  keyT[i][:, t16 * 128:(t16 + 1) * 128],
                                  qry[:, i, :],
                                  start=(i == 0), stop=(i == KC - 1),
                              )
                          nc.scalar.activation(exp_blk[:, t16, :], psum[:], AF.Exp)
                          half = t16 // 8
                          if t16 % 8 == 1:
                              nc.vector.tensor_add(
                                  facc[half][:], exp_blk[:, t16 - 1, :],
                                  exp_blk[:, t16, :],
                              )
                          elif t16 % 8 > 1:
                              nc.vector.tensor_add(
                                  facc[half][:], facc[half][:],
                                  exp_blk[:, t16, :],
                              )

                      # column sums over all sk (partition dim): 2 ones-matmuls
                      sums_ps = ps1pool.tile([1, 512], F32, tag="ps1")
                      nc.tensor.matmul(sums_ps[:], ones[:], facc[0][:],
                                       start=True, stop=False)
                      nc.tensor.matmul(sums_ps[:], ones[:], facc[1][:],
                                       start=False, stop=True)
                      sums_sb = sumpool.tile([1, 512], F32, tag="sums")
                      nc.vector.tensor_copy(sums_sb[:], sums_ps[:])

                      # r = 1/sums as per-partition scalars, via [1,128] PE
                      # transpose; emitted before UT so its PE<->DVE chain is
                      # hidden under the UT matmul stream
                      r_sb = rpool.tile([128, 4], F32, tag="r")
                      for m in range(4):
                          pr = psrpool.tile([128, 1], F32, tag="psr")
                          nc.tensor.transpose(
                              pr[:], sums_sb[0:1, m * 128:(m + 1) * 128], ident[:]
                          )
                          nc.vector.reciprocal(r_sb[:, m:m + 1], pr[:])

                      # UT block [d, 512] = value.T @ expT
                      ut = upool.tile([128, KC, 512], BF16, tag="UT")
                      for j in range(KC):
                          psum = pspool.tile([128, 512], F32, tag="ps")
                          for t16 in range(ST):
                              nc.tensor.matmul(
                                  psum[:],
                                  val[:, t16, j * 128:(j + 1) * 128],
                                  exp_blk[:, t16, :],
                                  start=(t16 == 0), stop=(t16 == ST - 1),
                              )
                          nc.vector.tensor_copy(ut[:, j, :], psum[:])

                      # final block: out[sq, d] = (UT.T @ Wo) * r + bo
                      for m in range(4):
                          ob = opool.tile([128, D], F32, tag="outb")
                          sq = blk * 512 + m * 128
                          # split the very last store per 512-half so the
                          # first half ships while the second computes
                          last = (b == NB - 1) and (blk == NBLK - 1) and (m == 3)
                          for n in range(2):
                              psum = pspool.tile([128, 512], F32, tag="ps")
                              for j in range(KC):
                                  nc.tensor.matmul(
                                      psum[:],
                                      ut[:, j, m * 128:(m + 1) * 128],
                                      wo[:, j, n * 512:(n + 1) * 512],
                                      start=(j == 0), stop=(j == KC - 1),
                                  )
                              nc.vector.tensor_scalar_mul(
                                  ob[:, n * 512:(n + 1) * 512], psum[:], r_sb[:, m:m + 1]
                              )
                              nc.vector.tensor_add(
                                  ob[:, n * 512:(n + 1) * 512],
                                  ob[:, n * 512:(n + 1) * 512],
                                  bo_sb[:, n * 512:(n + 1) * 512],
                              )
                              if last:
                                  nc.sync.dma_start(
                                      out=out_d[b, sq:sq + 128, n * 512:(n + 1) * 512],
                                      in_=ob[:, n * 512:(n + 1) * 512],
                                  )
                          if not last:
                              nc.sync.dma_start(out=out_d[b, sq:sq + 128, :], in_=ob[:])

    if reps == 1:
        _strip_dead_pe_updates(nc)
    _split_waits(nc)
    return nc


_PROGRAM = None


def _get_program():
    global _PROGRAM
    if _PROGRAM is None:
        _PROGRAM = build_program()
    return _PROGRAM


def prepare_in_maps(q, k, v, Wq, bq, Wk, bk, Wv, bv, Wo, bo):
    bf = ml_dtypes.bfloat16
    f32 = np.float32

    def t_bf16(x):  # [B,S,D] f32 -> [B,D,S] bf16 contiguous
        return np.ascontiguousarray(
            np.asarray(x, f32).astype(bf).transpose(0, 2, 1)
        )

    qT = t_bf16(q)
    kT = t_bf16(k)
    vT = t_bf16(v)
    Wq_b = np.asarray(Wq, f32).astype(bf)
    Wk_b = np.asarray(Wk, f32).astype(bf)
    Wv_b = np.asarray(Wv, f32).astype(bf)
    Wo_b = np.asarray(Wo, f32).astype(bf)
    bq2 = np.ascontiguousarray(
        (np.asarray(bq, f32) * np.float32(SCALE)).reshape(KC, 128).T
    )
    bk2 = np.ascontiguousarray(np.asarray(bk, f32).reshape(KC, 128).T)
    bv1 = np.ascontiguousarray(np.asarray(bv, f32)).astype(bf)
    bo1 = np.ascontiguousarray(np.asarray(bo, f32)).astype(bf)

    in_maps = []
    for c in range(N_CORES):
        sl = slice(c * NB, (c + 1) * NB)
        in_maps.append({
            "qT": qT[sl], "kT": kT[sl], "vT": vT[sl],
            "Wq": Wq_b, "Wk": Wk_b, "Wv": Wv_b, "Wo": Wo_b,
            "bq": bq2, "bk": bk2, "bv": bv1, "bo": bo1,
        })
    return in_maps


def kernel(q, k, v, Wq, bq, Wk, bk, Wv, bv, Wo, bo):
    nc = _get_program()
    in_maps = prepare_in_maps(q, k, v, Wq, bq, Wk, bk, Wv, bv, Wo, bo)
    res = run_bass_kernel_spmd(nc, in_maps, core_ids=list(range(N_CORES)))
    out = np.concatenate([res.results[c]["out"] for c in range(N_CORES)], axis=0)
    return out.astype(np.float32)


# revision 10
# speedup vs baseline: 1.0476x; 1.0014x over previous
"""CrossAttention (single-head) Trainium2 kernel, 8-core data-parallel.

Full inputs in, full output out. Internally: batch 16 is sharded 2-per-core
across 8 NeuronCores; each core runs the whole attention layer for its two
batches in bf16 (f32 PSUM accumulation), with activations kept in transposed
[d, s] layout so every matmul contracts over the partition dim without any
on-chip transposes of large tensors.

v2 perf changes vs baseline:
- All multi-tile loads (weights, per-block activations) are single-trigger
  3D-AP DMAs: fewer Sync-engine trigger slots, less HWDGE FIFO serialization.
- DMA emission order puts batch-0 K-projection inputs (kin, Wk halves, bk)
  first so the first matmul starts ~9us in instead of ~31us.
- Softmax denominators: the 16 exp tiles are pair-folded on the Vector
  engine down to 2 accumulators, so the partition-sum ones-matmul chain is
  2 matmuls per block instead of 16 (saves ~24us of PE time).
"""

import sys

sys.path.insert(0, "/opt/trn_rl_repo")

import numpy as np
import ml_dtypes

import concourse.bass as bass
import concourse.mybir as mybir
import concourse.tile as tile
from concourse.bass_utils import run_bass_kernel_spmd

BF16 = mybir.dt.bfloat16
F32 = mybir.dt.float32
AF = mybir.ActivationFunctionType

N_CORES = 8
B, S, D = 16, 2048, 1024
NB = B // N_CORES          # batches per core
KC = D // 128              # 8 chunks of 128 along d
ST = S // 128              # 16 tiles of 128 along s
NBLK = S // 512            # 4 blocks of 512 along s
SCALE = 1.0 / np.sqrt(np.float32(D))  # 1/32


def _split_waits(nc, limit=1):
    """Walrus in this container allows at most one sync wait per instruction:
    hoist excess waits onto NoOp carriers inserted just before."""
    n_new = 0
    for f in nc.m.functions:
        for bb in f.blocks:
            new_insts = []
            for inst in bb.instructions:
                si = inst.sync_info
                waits = list(si.on_wait) if si and si.on_wait else []
                if len(waits) > limit:
                    excess, keep = waits[:-limit], waits[-limit:]
                    for i in range(0, len(excess), limit):
                        chunk = excess[i:i + limit]
                        nop = mybir.InstNoOp(
                            name=f"{inst.name}-ws-{n_new}",
                            ins=[], outs=[],
                            sync_info=mybir.SyncInfo(on_wait=chunk, on_update=[]),
                        )
                        nop.engine = inst.engine
                        new_insts.append(nop)
                        n_new += 1
                    si.on_wait = keep
                new_insts.append(inst)
            bb.instructions[:] = new_insts
    return n_new


def _strip_dead_pe_updates(nc):
    """Drop PE sem increments nobody waits on (Tile emits one per matmul;
    only group-stop indices are ever waited). Renumber wait thresholds by
    rank among kept updates — release timing is identical, PE saves ~26ns
    per dropped serialized EVT_SEM write. Straight-line programs only."""
    pe = mybir.EngineType.PE
    insts = [i for f in nc.m.functions for bb in f.blocks for i in bb.instructions]
    upd_by_sem, wait_by_sem, bad = {}, {}, set()
    for inst in insts:
        si = inst.sync_info
        if not si:
            continue
        for u in (si.on_update or []):
            if u.sync_type != "semaphore":
                continue
            if inst.engine != pe or u.update_mode != "sem-inc" or u.update_value != 1:
                bad.add(u.id)
            upd_by_sem.setdefault(u.id, []).append((inst, u))
        for w in (si.on_wait or []):
            if w.sync_type != "semaphore":
                continue
            if w.wait_mode != "sem-ge-imm" or w.wait_reg is not None:
                bad.add(w.id)
            wait_by_sem.setdefault(w.id, []).append(w)
    n_drop = 0
    for sem_id, ups in upd_by_sem.items():
        if sem_id in bad or sem_id not in wait_by_sem or len(ups) < 16:
            continue
        waited = sorted({w.wait_value for w in wait_by_sem[sem_id]})
        if not waited or waited[-1] > len(ups) or waited[0] < 1:
            continue
        keep = set(waited)
        rank = {t: k + 1 for k, t in enumerate(waited)}
        for idx, (inst, u) in enumerate(ups, start=1):
            if idx not in keep:
                inst.sync_info.on_update = [
                    x for x in inst.sync_info.on_update if x is not u
                ]
                n_drop += 1
        for w in wait_by_sem[sem_id]:
            w.wait_value = rank[w.wait_value]
    return n_drop


def build_program(reps=1):
    nc = bass.Bass()

    qT_d = nc.declare_dram_parameter("qT", [NB, D, S], BF16, isOutput=False)
    kT_d = nc.declare_dram_parameter("kT", [NB, D, S], BF16, isOutput=False)
    vT_d = nc.declare_dram_parameter("vT", [NB, D, S], BF16, isOutput=False)
    Wq_d = nc.declare_dram_parameter("Wq", [D, D], BF16, isOutput=False)
    Wk_d = nc.declare_dram_parameter("Wk", [D, D], BF16, isOutput=False)
    Wv_d = nc.declare_dram_parameter("Wv", [D, D], BF16, isOutput=False)
    Wo_d = nc.declare_dram_parameter("Wo", [D, D], BF16, isOutput=False)
    # bq pre-scaled by 1/32 and reshaped [128, KC] host-side; bk likewise unscaled
    bq_d = nc.declare_dram_parameter("bq", [128, KC], F32, isOutput=False)
    bk_d = nc.declare_dram_parameter("bk", [128, KC], F32, isOutput=False)
    bv_d = nc.declare_dram_parameter("bv", [D], BF16, isOutput=False)
    bo_d = nc.declare_dram_parameter("bo", [D], BF16, isOutput=False)
    out_d = nc.declare_dram_parameter("out", [NB, S, D], F32, isOutput=True)

    def w_ap(w_d, col0, ncol):
        """[D, D] weight -> SBUF [128, KC, ncol] chunk-major AP (cols
        col0:col0+ncol of every 128-row chunk) in one DMA."""
        ap = w_d[:]
        return bass.AP(
            tensor=ap.tensor, offset=ap.offset + col0,
            ap=[[D, 128], [128 * D, KC], [1, ncol]],
        )

    def x_ap(x_d, b, s0, ncol):
        """[NB, D, S] activation -> SBUF [128, KC, ncol] chunk-major AP."""
        ap = x_d[:]
        return bass.AP(
            tensor=ap.tensor, offset=ap.offset + b * D * S + s0,
            ap=[[S, 128], [128 * S, KC], [1, ncol]],
        )

    def bcast_ap(v_d):
        ap = v_d[:]
        return bass.AP(tensor=ap.tensor, offset=ap.offset, ap=[[0, 128]] + ap.ap)

    from contextlib import ExitStack
    with tile.TileContext(nc) as tc:
        with ExitStack() as _stk:
            _p = lambda **kw: _stk.enter_context(tc.tile_pool(**kw))
            wqopool = _p(name="wqo", bufs=1)
            wkvpool = _p(name="wkv", bufs=2)
            inpool = _p(name="inp", bufs=3)
            kpool = _p(name="keyT", bufs=8)
            vpool = _p(name="value", bufs=1)
            qpool = _p(name="queryT", bufs=1)
            epool = _p(name="expT", bufs=1)
            fpool = _p(name="fold", bufs=2)
            upool = _p(name="UT", bufs=1)
            opool = _p(name="outb", bufs=2)
            sumpool = _p(name="sums", bufs=2)
            rpool = _p(name="rpool", bufs=2)
            cpool = _p(name="const", bufs=1)
            pspool = _p(name="ps", bufs=5, space="PSUM")
            ps1pool = _p(name="ps1", bufs=1, space="PSUM")
            psrpool = _p(name="psr", bufs=2, space="PSUM")

            # constants (cheap memsets; no DMA)
            ones = cpool.tile([128, 1], BF16, tag="ones")
            nc.vector.memset(ones[:], 1.0)
            ident = cpool.tile([1, 1], F32, tag="ident")
            nc.vector.memset(ident[:], 1.0)

            # ~4us of dummy matmuls fill the initial DMA wait and warm the
            # PE clock gate (HAM) so the real stream starts at 2.4 GHz
            warm_ps = psrpool.tile([1, 1], F32, tag="psr", name="warm")
            for _ in range(40):
                nc.tensor.matmul(warm_ps[:], ones[:, 0:1], ones[:, 0:1],
                                 start=True, stop=True)

            # ---- batch-0 critical-path DMAs first: kin(sblk0), Wk, bk ----
            # Split so the first chain's first-half accumulation (kin chunks
            # 0-3 x Wk chunks 0-3) can start after ~1MB of transfer; subtile
            # deps gate each matmul on just the DMA covering its region.
            kin0 = inpool.tile([128, KC, 512], BF16, tag="inp", name="kin0")
            wk_b = [None] * NB
            wv_b = [None] * NB
            wk_b[0] = wkvpool.tile([128, KC, D], BF16, tag="wkv", name="wk0")

            def half_x_ap(x_d, b, s0, ch0):
                ap = x_d[:]
                return bass.AP(
                    tensor=ap.tensor,
                    offset=ap.offset + b * D * S + ch0 * 128 * S + s0,
                    ap=[[S, 128], [128 * S, KC // 2], [1, 512]],
                )

            def half_w_ap(w_d, col0, ncol, ch0):
                ap = w_d[:]
                return bass.AP(
                    tensor=ap.tensor, offset=ap.offset + ch0 * 128 * D + col0,
                    ap=[[D, 128], [128 * D, KC // 2], [1, ncol]],
                )

            nc.sync.dma_start(out=kin0[:, 0:4, :], in_=half_x_ap(kT_d, 0, 0, 0))
            nc.sync.dma_start(out=wk_b[0][:, 0:4, 0:512], in_=half_w_ap(Wk_d, 0, 512, 0))
            nc.sync.dma_start(out=kin0[:, 4:8, :], in_=half_x_ap(kT_d, 0, 0, 4))
            nc.sync.dma_start(out=wk_b[0][:, 4:8, 0:512], in_=half_w_ap(Wk_d, 0, 512, 4))
            nc.sync.dma_start(out=wk_b[0][:, :, 512:D], in_=w_ap(Wk_d, 512, 512))
            bk_sb = cpool.tile([128, KC], F32, tag="bk")
            nc.sync.dma_start(out=bk_sb[:], in_=bk_d[:])

            # deferred-load tiles (DMAs emitted mid-stream below)
            bq_sb = cpool.tile([128, KC], F32, tag="bq")
            bv_sb = cpool.tile([128, D], BF16, tag="bv")
            bo_sb = cpool.tile([128, D], BF16, tag="bo")
            wq = wqopool.tile([128, KC, D], BF16, tag="wq")
            wo = wqopool.tile([128, KC, D], BF16, tag="wo")

            import contextlib
            loop_ctx = tc.For_i(0, reps, 1) if reps > 1 else contextlib.nullcontext()
            with loop_ctx:
              for b in range(NB):
                  if b > 0:
                      wk_b[b] = wkvpool.tile([128, KC, D], BF16, tag="wkv",
                                             name=f"wk{b}")
                      nc.sync.dma_start(out=wk_b[b][:], in_=w_ap(Wk_d, 0, D))
                  wk = wk_b[b]

                  # ---------------- keyT[d, s] = Wk.T @ kT (+bk) ----------------
                  keyT = [kpool.tile([128, S], BF16, tag="keyT", name=f"keyT{i}")
                          for i in range(KC)]
                  for s in range(NBLK):
                      if b == 0 and s == 0:
                          kin = kin0
                      else:
                          kin = inpool.tile([128, KC, 512], BF16, tag="inp",
                                            name=f"kin{s}")
                          nc.sync.dma_start(out=kin[:], in_=x_ap(kT_d, b, s * 512, 512))
                      for do in range(KC):
                          psum = pspool.tile([128, 512], F32, tag="ps")
                          for i in range(KC):
                              nc.tensor.matmul(
                                  psum[:], wk[:, i, do * 128:(do + 1) * 128],
                                  kin[:, i, :],
                                  start=(i == 0), stop=(i == KC - 1),
                              )
                          nc.vector.tensor_scalar_add(
                              keyT[do][:, s * 512:(s + 1) * 512], psum[:],
                              bk_sb[:, do:do + 1],
                          )
                      if b == 0 and s == 0:
                          # Wv + bv arrive during remaining K-proj compute
                          wv_b[0] = wkvpool.tile([128, KC, D], BF16, tag="wkv",
                                                 name="wv0")
                          nc.sync.dma_start(out=wv_b[0][:], in_=w_ap(Wv_d, 0, D))
                          nc.sync.dma_start(out=bv_sb[:], in_=bcast_ap(bv_d))

                  if b > 0:
                      wv_b[b] = wkvpool.tile([128, KC, D], BF16, tag="wkv",
                                             name=f"wv{b}")
                      nc.sync.dma_start(out=wv_b[b][:], in_=w_ap(Wv_d, 0, D))
                  wv = wv_b[b]

                  # ---------------- value[s, d] = vT.T @ Wv (+bv) ----------------
                  val = vpool.tile([128, ST, D], BF16, tag="value")
                  for s in range(NBLK):
                      vin = inpool.tile([128, KC, 512], BF16, tag="inp",
                                        name=f"vin{s}")
                      nc.sync.dma_start(out=vin[:], in_=x_ap(vT_d, b, s * 512, 512))
                      for tt in range(4):
                          t16 = s * 4 + tt
                          for n in range(2):
                              psum = pspool.tile([128, 512], F32, tag="ps")
                              for i in range(KC):
                                  nc.tensor.matmul(
                                      psum[:],
                                      vin[:, i, tt * 128:(tt + 1) * 128],
                                      wv[:, i, n * 512:(n + 1) * 512],
                                      start=(i == 0), stop=(i == KC - 1),
                                  )
                              nc.vector.tensor_add(
                                  val[:, t16, n * 512:(n + 1) * 512], psum[:],
                                  bv_sb[:, n * 512:(n + 1) * 512],
                              )
                      if b == 0 and s == 0:
                          # Wq/bq land before the first qry block; Wo/bo are
                          # only read in the final phase, much later
                          nc.sync.dma_start(out=bq_sb[:], in_=bq_d[:])
                          nc.sync.dma_start(out=wq[:], in_=w_ap(Wq_d, 0, D))
                      if b == 0 and s == 2:
                          nc.sync.dma_start(out=wo[:], in_=w_ap(Wo_d, 0, D))
                          nc.sync.dma_start(out=bo_sb[:], in_=bcast_ap(bo_d))

                  # ---------------- per 512-wide sq block ----------------
                  for blk in range(NBLK):
                      # queryT block [d, 512] = Wq.T @ qT_blk, scaled 1/32 (+bq/32)
                      qin = inpool.tile([128, KC, 512], BF16, tag="inp",
                                        name=f"qin{blk}")
                      nc.sync.dma_start(out=qin[:], in_=x_ap(qT_d, b, blk * 512, 512))
                      qry = qpool.tile([128, KC, 512], BF16, tag="queryT")
                      for do in range(KC):
                          psum = pspool.tile([128, 512], F32, tag="ps")
                          for i in range(KC):
                              nc.tensor.matmul(
                                  psum[:], wq[:, i, do * 128:(do + 1) * 128],
                                  qin[:, i, :],
                                  start=(i == 0), stop=(i == KC - 1),
                              )
                          nc.vector.tensor_scalar(
                              out=qry[:, do, :], in0=psum[:], scalar1=float(SCALE),
                              scalar2=bq_sb[:, do:do + 1],
                              op0=mybir.AluOpType.mult, op1=mybir.AluOpType.add,
                          )

                      # scoresT -> expT, with pairwise DVE fold of exp tiles
                      # into 2 accumulators for the partition-sum
                      exp_blk = epool.tile([128, ST, 512], BF16, tag="expT")
                      facc = [
                          fpool.tile([128, 512], BF16, tag="fold", name="facc0"),
                          fpool.tile([128, 512], BF16, tag="fold", name="facc1"),
                      ]
                      for t16 in range(ST):
                          psum = pspool.tile([128, 512], F32, tag="ps")
                          for i in range(KC):
                              nc.tensor.matmul(
                                  psum[:],
                                  keyT[i][:, t16 * 128:(t16 + 1) * 128],
                                  qry[:, i, :],
                                  start=(i == 0), stop=(i == KC - 1),
                              )
                          nc.scalar.activation(exp_blk[:, t16, :], psum[:], AF.Exp)
                          half = t16 // 8
                          if t16 % 8 == 1:
                              nc.vector.tensor_add(
                                  facc[half][:], exp_blk[:, t16 - 1, :],
                                  exp_blk[:, t16, :],
                              )
                          elif t16 % 8 > 1:
                              nc.vector.tensor_add(
                                  facc[half][:], facc[half][:],
                                  exp_blk[:, t16, :],
                              )

                      # column sums over all sk (partition dim): 2 ones-matmuls
                      sums_ps = ps1pool.tile([1, 512], F32, tag="ps1")
                      nc.tensor.matmul(sums_ps[:], ones[:], facc[0][:],
                                       start=True, stop=False)
                      nc.tensor.matmul(sums_ps[:], ones[:], facc[1][:],
                                       start=False, stop=True)
                      sums_sb = sumpool.tile([1, 512], F32, tag="sums")
                      nc.vector.tensor_copy(sums_sb[:], sums_ps[:])

                      # r = 1/sums as per-partition scalars, via [1,128] PE
                      # transpose; emitted before UT so its PE<->DVE chain is
                      # hidden under the UT matmul stream
                      r_sb = rpool.tile([128, 4], F32, tag="r")
                      for m in range(4):
                          pr = psrpool.tile([128, 1], F32, tag="psr")
                          nc.tensor.transpose(
                              pr[:], sums_sb[0:1, m * 128:(m + 1) * 128], ident[:]
                          )
                          nc.vector.reciprocal(r_sb[:, m:m + 1], pr[:])

                      # UT block [d, 512] = value.T @ expT
                      ut = upool.tile([128, KC, 512], BF16, tag="UT")
                      for j in range(KC):
                          psum = pspool.tile([128, 512], F32, tag="ps")
                          for t16 in range(ST):
                              nc.tensor.matmul(
                                  psum[:],
                                  val[:, t16, j * 128:(j + 1) * 128],
                                  exp_blk[:, t16, :],
                                  start=(t16 == 0), stop=(t16 == ST - 1),
                              )
                          nc.vector.tensor_copy(ut[:, j, :], psum[:])

                      # final block: out[sq, d] = (UT.T @ Wo) * r + bo
                      for m in range(4):
                          ob = opool.tile([128, D], F32, tag="outb")
                          sq = blk * 512 + m * 128
                          # split the very last store per 512-half so the
                          # first half ships while the second computes
                          last = (b == NB - 1) and (blk == NBLK - 1) and (m == 3)
                          for n in range(2):
                              psum = pspool.tile([128, 512], F32, tag="ps")
                              for j in range(KC):
                                  nc.tensor.matmul(
                                      psum[:],
                                      ut[:, j, m * 128:(m + 1) * 128],
                                      wo[:, j, n * 512:(n + 1) * 512],
                                      start=(j == 0), stop=(j == KC - 1),
                                  )
                              # ob = (psum * r) + bo in one fused DVE op;
                              # the very last half goes in 256-wide pieces so
                              # compute/store pipeline to the end
                              pieces = 2 if (last and n == 1) else 1
                              for p in range(pieces):
                                  w = 512 // pieces
                                  c0 = n * 512 + p * w
                                  nc.vector.scalar_tensor_tensor(
                                      out=ob[:, c0:c0 + w],
                                      in0=psum[:, p * w:(p + 1) * w],
                                      scalar=r_sb[:, m:m + 1],
                                      in1=bo_sb[:, c0:c0 + w],
                                      op0=mybir.AluOpType.mult,
                                      op1=mybir.AluOpType.add,
                                  )
                                  if last:
                                      nc.sync.dma_start(
                                          out=out_d[b, sq:sq + 128, c0:c0 + w],
                                          in_=ob[:, c0:c0 + w],
                                      )
                          if not last:
                              nc.sync.dma_start(out=out_d[b, sq:sq + 128, :], in_=ob[:])

    if reps == 1:
        _strip_dead_pe_updates(nc)
    _split_waits(nc)
    return nc


_PROGRAM = None


def _get_program():
    global _PROGRAM
    if _PROGRAM is None:
        _PROGRAM = build_program()
    return _PROGRAM


def prepare_in_maps(q, k, v, Wq, bq, Wk, bk, Wv, bv, Wo, bo):
    bf = ml_dtypes.bfloat16
    f32 = np.float32

    def t_bf16(x):  # [B,S,D] f32 -> [B,D,S] bf16 contiguous
        return np.ascontiguousarray(
            np.asarray(x, f32).astype(bf).transpose(0, 2, 1)
        )

    qT = t_bf16(q)
    kT = t_bf16(k)
    vT = t_bf16(v)
    Wq_b = np.asarray(Wq, f32).astype(bf)
    Wk_b = np.asarray(Wk, f32).astype(bf)
    Wv_b = np.asarray(Wv, f32).astype(bf)
    Wo_b = np.asarray(Wo, f32).astype(bf)
    bq2 = np.ascontiguousarray(
        (np.asarray(bq, f32) * np.float32(SCALE)).reshape(KC, 128).T
    )
    bk2 = np.ascontiguousarray(np.asarray(bk, f32).reshape(KC, 128).T)
    bv1 = np.ascontiguousarray(np.asarray(bv, f32)).astype(bf)
    bo1 = np.ascontiguousarray(np.asarray(bo, f32)).astype(bf)

    in_maps = []
    for c in range(N_CORES):
        sl = slice(c * NB, (c + 1) * NB)
        in_maps.append({
            "qT": qT[sl], "kT": kT[sl], "vT": vT[sl],
            "Wq": Wq_b, "Wk": Wk_b, "Wv": Wv_b, "Wo": Wo_b,
            "bq": bq2, "bk": bk2, "bv": bv1, "bo": bo1,
        })
    return in_maps


def kernel(q, k, v, Wq, bq, Wk, bk, Wv, bv, Wo, bo):
    nc = _get_program()
    in_maps = prepare_in_maps(q, k, v, Wq, bq, Wk, bk, Wv, bv, Wo, bo)
    res = run_bass_kernel_spmd(nc, in_maps, core_ids=list(range(N_CORES)))
    out = np.concatenate([res.results[c]["out"] for c in range(N_CORES)], axis=0)
    return out.astype(np.float32)


# revision 11
# speedup vs baseline: 1.0476x; 1.0000x over previous
"""CrossAttention (single-head) Trainium2 kernel, 8-core data-parallel.

Full inputs in, full output out. Internally: batch 16 is sharded 2-per-core
across 8 NeuronCores; each core runs the whole attention layer for its two
batches in bf16 (f32 PSUM accumulation), with activations kept in transposed
[d, s] layout so every matmul contracts over the partition dim without any
on-chip transposes of large tensors.

v2 perf changes vs baseline:
- All multi-tile loads (weights, per-block activations) are single-trigger
  3D-AP DMAs: fewer Sync-engine trigger slots, less HWDGE FIFO serialization.
- DMA emission order puts batch-0 K-projection inputs (kin, Wk halves, bk)
  first so the first matmul starts ~9us in instead of ~31us.
- Softmax denominators: the 16 exp tiles are pair-folded on the Vector
  engine down to 2 accumulators, so the partition-sum ones-matmul chain is
  2 matmuls per block instead of 16 (saves ~24us of PE time).
"""

import sys

sys.path.insert(0, "/opt/trn_rl_repo")

import numpy as np
import ml_dtypes

import concourse.bass as bass
import concourse.mybir as mybir
import concourse.tile as tile
from concourse.bass_utils import run_bass_kernel_spmd

BF16 = mybir.dt.bfloat16
F32 = mybir.dt.float32
AF = mybir.ActivationFunctionType

N_CORES = 8
B, S, D = 16, 2048, 1024
NB = B // N_CORES          # batches per core
KC = D // 128              # 8 chunks of 128 along d
ST = S // 128              # 16 tiles of 128 along s
NBLK = S // 512            # 4 blocks of 512 along s
SCALE = 1.0 / np.sqrt(np.float32(D))  # 1/32


def _split_waits(nc, limit=1):
    """Walrus in this container allows at most one sync wait per instruction:
    hoist excess waits onto NoOp carriers inserted just before."""
    n_new = 0
    for f in nc.m.functions:
        for bb in f.blocks:
            new_insts = []
            for inst in bb.instructions:
                si = inst.sync_info
                waits = list(si.on_wait) if si and si.on_wait else []
                if len(waits) > limit:
                    excess, keep = waits[:-limit], waits[-limit:]
                    for i in range(0, len(excess), limit):
                        chunk = excess[i:i + limit]
                        nop = mybir.InstNoOp(
                            name=f"{inst.name}-ws-{n_new}",
                            ins=[], outs=[],
                            sync_info=mybir.SyncInfo(on_wait=chunk, on_update=[]),
                        )
                        nop.engine = inst.engine
                        new_insts.append(nop)
                        n_new += 1
                    si.on_wait = keep
                new_insts.append(inst)
            bb.instructions[:] = new_insts
    return n_new


def _strip_dead_pe_updates(nc):
    """Drop PE sem increments nobody waits on (Tile emits one per matmul;
    only group-stop indices are ever waited). Renumber wait thresholds by
    rank among kept updates — release timing is identical, PE saves ~26ns
    per dropped serialized EVT_SEM write. Straight-line programs only."""
    pe = mybir.EngineType.PE
    insts = [i for f in nc.m.functions for bb in f.blocks for i in bb.instructions]
    upd_by_sem, wait_by_sem, bad = {}, {}, set()
    for inst in insts:
        si = inst.sync_info
        if not si:
            continue
        for u in (si.on_update or []):
            if u.sync_type != "semaphore":
                continue
            if inst.engine != pe or u.update_mode != "sem-inc" or u.update_value != 1:
                bad.add(u.id)
            upd_by_sem.setdefault(u.id, []).append((inst, u))
        for w in (si.on_wait or []):
            if w.sync_type != "semaphore":
                continue
            if w.wait_mode != "sem-ge-imm" or w.wait_reg is not None:
                bad.add(w.id)
            wait_by_sem.setdefault(w.id, []).append(w)
    n_drop = 0
    for sem_id, ups in upd_by_sem.items():
        if sem_id in bad or sem_id not in wait_by_sem or len(ups) < 16:
            continue
        waited = sorted({w.wait_value for w in wait_by_sem[sem_id]})
        if not waited or waited[-1] > len(ups) or waited[0] < 1:
            continue
        keep = set(waited)
        rank = {t: k + 1 for k, t in enumerate(waited)}
        for idx, (inst, u) in enumerate(ups, start=1):
            if idx not in keep:
                inst.sync_info.on_update = [
                    x for x in inst.sync_info.on_update if x is not u
                ]
                n_drop += 1
        for w in wait_by_sem[sem_id]:
            w.wait_value = rank[w.wait_value]
    return n_drop


def build_program(reps=1):
    nc = bass.Bass()

    qT_d = nc.declare_dram_parameter("qT", [NB, D, S], BF16, isOutput=False)
    kT_d = nc.declare_dram_parameter("kT", [NB, D, S], BF16, isOutput=False)
    vT_d = nc.declare_dram_parameter("vT", [NB, D, S], BF16, isOutput=False)
    Wq_d = nc.declare_dram_parameter("Wq", [D, D], BF16, isOutput=False)
    Wk_d = nc.declare_dram_parameter("Wk", [D, D], BF16, isOutput=False)
    Wv_d = nc.declare_dram_parameter("Wv", [D, D], BF16, isOutput=False)
    Wo_d = nc.declare_dram_parameter("Wo", [D, D], BF16, isOutput=False)
    # bq pre-scaled by 1/32 and reshaped [128, KC] host-side; bk likewise unscaled
    bq_d = nc.declare_dram_parameter("bq", [128, KC], F32, isOutput=False)
    bk_d = nc.declare_dram_parameter("bk", [128, KC], F32, isOutput=False)
    bv_d = nc.declare_dram_parameter("bv", [D], BF16, isOutput=False)
    bo_d = nc.declare_dram_parameter("bo", [D], BF16, isOutput=False)
    out_d = nc.declare_dram_parameter("out", [NB, S, D], F32, isOutput=True)

    def w_ap(w_d, col0, ncol):
        """[D, D] weight -> SBUF [128, KC, ncol] chunk-major AP (cols
        col0:col0+ncol of every 128-row chunk) in one DMA."""
        ap = w_d[:]
        return bass.AP(
            tensor=ap.tensor, offset=ap.offset + col0,
            ap=[[D, 128], [128 * D, KC], [1, ncol]],
        )

    def x_ap(x_d, b, s0, ncol):
        """[NB, D, S] activation -> SBUF [128, KC, ncol] chunk-major AP."""
        ap = x_d[:]
        return bass.AP(
            tensor=ap.tensor, offset=ap.offset + b * D * S + s0,
            ap=[[S, 128], [128 * S, KC], [1, ncol]],
        )

    def bcast_ap(v_d):
        ap = v_d[:]
        return bass.AP(tensor=ap.tensor, offset=ap.offset, ap=[[0, 128]] + ap.ap)

    from contextlib import ExitStack
    with tile.TileContext(nc) as tc:
        with ExitStack() as _stk:
            _p = lambda **kw: _stk.enter_context(tc.tile_pool(**kw))
            wqopool = _p(name="wqo", bufs=1)
            wkvpool = _p(name="wkv", bufs=2)
            inpool = _p(name="inp", bufs=3)
            kpool = _p(name="keyT", bufs=8)
            vpool = _p(name="value", bufs=1)
            qpool = _p(name="queryT", bufs=1)
            epool = _p(name="expT", bufs=1)
            fpool = _p(name="fold", bufs=2)
            upool = _p(name="UT", bufs=1)
            opool = _p(name="outb", bufs=2)
            sumpool = _p(name="sums", bufs=2)
            rpool = _p(name="rpool", bufs=2)
            cpool = _p(name="const", bufs=1)
            pspool = _p(name="ps", bufs=5, space="PSUM")
            ps1pool = _p(name="ps1", bufs=1, space="PSUM")
            psrpool = _p(name="psr", bufs=2, space="PSUM")

            # constants (cheap memsets; no DMA)
            ones = cpool.tile([128, 1], BF16, tag="ones")
            nc.vector.memset(ones[:], 1.0)
            ident = cpool.tile([1, 1], F32, tag="ident")
            nc.vector.memset(ident[:], 1.0)

            # ~6us of dummy matmuls fill the initial DMA wait and warm the
            # PE clock gate (HAM) so the real stream starts at 2.4 GHz
            wtile = cpool.tile([128, 128], BF16, tag="warm")
            nc.vector.memset(wtile[:], 0.0)
            warm_ps = psrpool.tile([1, 128], F32, tag="psr", name="warm")
            for _ in range(60):
                nc.tensor.matmul(warm_ps[:], ones[:, 0:1], wtile[:],
                                 start=True, stop=True)

            # ---- batch-0 critical-path DMAs first: kin(sblk0), Wk, bk ----
            # Split so the first chain's first-half accumulation (kin chunks
            # 0-3 x Wk chunks 0-3) can start after ~1MB of transfer; subtile
            # deps gate each matmul on just the DMA covering its region.
            kin0 = inpool.tile([128, KC, 512], BF16, tag="inp", name="kin0")
            wk_b = [None] * NB
            wv_b = [None] * NB
            wk_b[0] = wkvpool.tile([128, KC, D], BF16, tag="wkv", name="wk0")

            def half_x_ap(x_d, b, s0, ch0):
                ap = x_d[:]
                return bass.AP(
                    tensor=ap.tensor,
                    offset=ap.offset + b * D * S + ch0 * 128 * S + s0,
                    ap=[[S, 128], [128 * S, KC // 2], [1, 512]],
                )

            def half_w_ap(w_d, col0, ncol, ch0):
                ap = w_d[:]
                return bass.AP(
                    tensor=ap.tensor, offset=ap.offset + ch0 * 128 * D + col0,
                    ap=[[D, 128], [128 * D, KC // 2], [1, ncol]],
                )

            nc.sync.dma_start(out=kin0[:, 0:4, :], in_=half_x_ap(kT_d, 0, 0, 0))
            nc.sync.dma_start(out=wk_b[0][:, 0:4, 0:512], in_=half_w_ap(Wk_d, 0, 512, 0))
            nc.sync.dma_start(out=kin0[:, 4:8, :], in_=half_x_ap(kT_d, 0, 0, 4))
            nc.sync.dma_start(out=wk_b[0][:, 4:8, 0:512], in_=half_w_ap(Wk_d, 0, 512, 4))
            nc.sync.dma_start(out=wk_b[0][:, :, 512:D], in_=w_ap(Wk_d, 512, 512))
            bk_sb = cpool.tile([128, KC], F32, tag="bk")
            nc.sync.dma_start(out=bk_sb[:], in_=bk_d[:])

            # deferred-load tiles (DMAs emitted mid-stream below)
            bq_sb = cpool.tile([128, KC], F32, tag="bq")
            bv_sb = cpool.tile([128, D], BF16, tag="bv")
            bo_sb = cpool.tile([128, D], BF16, tag="bo")
            wq = wqopool.tile([128, KC, D], BF16, tag="wq")
            wo = wqopool.tile([128, KC, D], BF16, tag="wo")

            import contextlib
            loop_ctx = tc.For_i(0, reps, 1) if reps > 1 else contextlib.nullcontext()
            with loop_ctx:
              for b in range(NB):
                  if b > 0:
                      wk_b[b] = wkvpool.tile([128, KC, D], BF16, tag="wkv",
                                             name=f"wk{b}")
                      nc.sync.dma_start(out=wk_b[b][:], in_=w_ap(Wk_d, 0, D))
                  wk = wk_b[b]

                  # ---------------- keyT[d, s] = Wk.T @ kT (+bk) ----------------
                  keyT = [kpool.tile([128, S], BF16, tag="keyT", name=f"keyT{i}")
                          for i in range(KC)]
                  for s in range(NBLK):
                      if b == 0 and s == 0:
                          kin = kin0
                      else:
                          kin = inpool.tile([128, KC, 512], BF16, tag="inp",
                                            name=f"kin{s}")
                          nc.sync.dma_start(out=kin[:], in_=x_ap(kT_d, b, s * 512, 512))
                      for do in range(KC):
                          psum = pspool.tile([128, 512], F32, tag="ps")
                          for i in range(KC):
                              nc.tensor.matmul(
                                  psum[:], wk[:, i, do * 128:(do + 1) * 128],
                                  kin[:, i, :],
                                  start=(i == 0), stop=(i == KC - 1),
                              )
                          nc.vector.tensor_scalar_add(
                              keyT[do][:, s * 512:(s + 1) * 512], psum[:],
                              bk_sb[:, do:do + 1],
                          )
                      if b == 0 and s == 0:
                          # Wv + bv arrive during remaining K-proj compute
                          wv_b[0] = wkvpool.tile([128, KC, D], BF16, tag="wkv",
                                                 name="wv0")
                          nc.sync.dma_start(out=wv_b[0][:], in_=w_ap(Wv_d, 0, D))
                          nc.sync.dma_start(out=bv_sb[:], in_=bcast_ap(bv_d))

                  if b > 0:
                      wv_b[b] = wkvpool.tile([128, KC, D], BF16, tag="wkv",
                                             name=f"wv{b}")
                      nc.sync.dma_start(out=wv_b[b][:], in_=w_ap(Wv_d, 0, D))
                  wv = wv_b[b]

                  # ---------------- value[s, d] = vT.T @ Wv (+bv) ----------------
                  val = vpool.tile([128, ST, D], BF16, tag="value")
                  for s in range(NBLK):
                      vin = inpool.tile([128, KC, 512], BF16, tag="inp",
                                        name=f"vin{s}")
                      nc.sync.dma_start(out=vin[:], in_=x_ap(vT_d, b, s * 512, 512))
                      for tt in range(4):
                          t16 = s * 4 + tt
                          for n in range(2):
                              psum = pspool.tile([128, 512], F32, tag="ps")
                              for i in range(KC):
                                  nc.tensor.matmul(
                                      psum[:],
                                      vin[:, i, tt * 128:(tt + 1) * 128],
                                      wv[:, i, n * 512:(n + 1) * 512],
                                      start=(i == 0), stop=(i == KC - 1),
                                  )
                              nc.vector.tensor_add(
                                  val[:, t16, n * 512:(n + 1) * 512], psum[:],
                                  bv_sb[:, n * 512:(n + 1) * 512],
                              )
                      if b == 0 and s == 0:
                          # Wq/bq land before the first qry block; Wo/bo are
                          # only read in the final phase, much later
                          nc.sync.dma_start(out=bq_sb[:], in_=bq_d[:])
                          nc.sync.dma_start(out=wq[:], in_=w_ap(Wq_d, 0, D))
                      if b == 0 and s == 2:
                          nc.sync.dma_start(out=wo[:], in_=w_ap(Wo_d, 0, D))
                          nc.sync.dma_start(out=bo_sb[:], in_=bcast_ap(bo_d))

                  # ---------------- per 512-wide sq block ----------------
                  for blk in range(NBLK):
                      # queryT block [d, 512] = Wq.T @ qT_blk, scaled 1/32 (+bq/32)
                      qin = inpool.tile([128, KC, 512], BF16, tag="inp",
                                        name=f"qin{blk}")
                      nc.sync.dma_start(out=qin[:], in_=x_ap(qT_d, b, blk * 512, 512))
                      qry = qpool.tile([128, KC, 512], BF16, tag="queryT")
                      for do in range(KC):
                          psum = pspool.tile([128, 512], F32, tag="ps")
                          for i in range(KC):
                              nc.tensor.matmul(
                                  psum[:], wq[:, i, do * 128:(do + 1) * 128],
                                  qin[:, i, :],
                                  start=(i == 0), stop=(i == KC - 1),
                              )
                          nc.vector.tensor_scalar(
                              out=qry[:, do, :], in0=psum[:], scalar1=float(SCALE),
                              scalar2=bq_sb[:, do:do + 1],
                              op0=mybir.AluOpType.mult, op1=mybir.AluOpType.add,
                          )

                      # scoresT -> expT, with pairwise DVE fold of exp tiles
                      # into 2 accumulators for the partition-sum
                      exp_blk = epool.tile([128, ST, 512], BF16, tag="expT")
                      facc = [
                          fpool.tile([128, 512], BF16, tag="fold", name="facc0"),
                          fpool.tile([128, 512], BF16, tag="fold", name="facc1"),
                      ]
                      for t16 in range(ST):
                          psum = pspool.tile([128, 512], F32, tag="ps")
                          for i in range(KC):
                              nc.tensor.matmul(
                                  psum[:],
                                  keyT[i][:, t16 * 128:(t16 + 1) * 128],
                                  qry[:, i, :],
                                  start=(i == 0), stop=(i == KC - 1),
                              )
                          nc.scalar.activation(exp_blk[:, t16, :], psum[:], AF.Exp)
                          half = t16 // 8
                          if t16 % 8 == 1:
                              nc.vector.tensor_add(
                                  facc[half][:], exp_blk[:, t16 - 1, :],
                                  exp_blk[:, t16, :],
                              )
                          elif t16 % 8 > 1:
                              nc.vector.tensor_add(
                                  facc[half][:], facc[half][:],
                                  exp_blk[:, t16, :],
                              )

                      # column sums over all sk (partition dim): 2 ones-matmuls
                      sums_ps = ps1pool.tile([1, 512], F32, tag="ps1")
                      nc.tensor.matmul(sums_ps[:], ones[:], facc[0][:],
                                       start=True, stop=False)
                      nc.tensor.matmul(sums_ps[:], ones[:], facc[1][:],
                                       start=False, stop=True)
                      sums_sb = sumpool.tile([1, 512], F32, tag="sums")
                      nc.vector.tensor_copy(sums_sb[:], sums_ps[:])

                      # r = 1/sums as per-partition scalars, via [1,128] PE
                      # transpose; emitted before UT so its PE<->DVE chain is
                      # hidden under the UT matmul stream
                      r_sb = rpool.tile([128, 4], F32, tag="r")
                      for m in range(4):
                          pr = psrpool.tile([128, 1], F32, tag="psr")
                          nc.tensor.transpose(
                              pr[:], sums_sb[0:1, m * 128:(m + 1) * 128], ident[:]
                          )
                          nc.vector.reciprocal(r_sb[:, m:m + 1], pr[:])

                      # UT block [d, 512] = value.T @ expT
                      ut = upool.tile([128, KC, 512], BF16, tag="UT")
                      for j in range(KC):
                          psum = pspool.tile([128, 512], F32, tag="ps")
                          for t16 in range(ST):
                              nc.tensor.matmul(
                                  psum[:],
                                  val[:, t16, j * 128:(j + 1) * 128],
                                  exp_blk[:, t16, :],
                                  start=(t16 == 0), stop=(t16 == ST - 1),
                              )
                          nc.vector.tensor_copy(ut[:, j, :], psum[:])

                      # final block: out[sq, d] = (UT.T @ Wo) * r + bo
                      for m in range(4):
                          ob = opool.tile([128, D], F32, tag="outb")
                          sq = blk * 512 + m * 128
                          # split the very last store per 512-half so the
                          # first half ships while the second computes
                          last = (b == NB - 1) and (blk == NBLK - 1) and (m == 3)
                          for n in range(2):
                              psum = pspool.tile([128, 512], F32, tag="ps")
                              for j in range(KC):
                                  nc.tensor.matmul(
                                      psum[:],
                                      ut[:, j, m * 128:(m + 1) * 128],
                                      wo[:, j, n * 512:(n + 1) * 512],
                                      start=(j == 0), stop=(j == KC - 1),
                                  )
                              # ob = (psum * r) + bo in one fused DVE op;
                              # the very last half goes in 256-wide pieces so
                              # compute/store pipeline to the end
                              pieces = 2 if (last and n == 1) else 1
                              for p in range(pieces):
                                  w = 512 // pieces
                                  c0 = n * 512 + p * w
                                  nc.vector.scalar_tensor_tensor(
                                      out=ob[:, c0:c0 + w],
                                      in0=psum[:, p * w:(p + 1) * w],
                                      scalar=r_sb[:, m:m + 1],
                                      in1=bo_sb[:, c0:c0 + w],
                                      op0=mybir.AluOpType.mult,
                                      op1=mybir.AluOpType.add,
                                  )
                                  if last:
                                      nc.sync.dma_start(
                                          out=out_d[b, sq:sq + 128, c0:c0 + w],
                                          in_=ob[:, c0:c0 + w],
                                      )
                          if not last:
                              nc.sync.dma_start(out=out_d[b, sq:sq + 128, :], in_=ob[:])

    if reps == 1:
        _strip_dead_pe_updates(nc)
    _split_waits(nc)
    return nc


_PROGRAM = None


def _get_program():
    global _PROGRAM
    if _PROGRAM is None:
        _PROGRAM = build_program()
    return _PROGRAM


def prepare_in_maps(q, k, v, Wq, bq, Wk, bk, Wv, bv, Wo, bo):
    bf = ml_dtypes.bfloat16
    f32 = np.float32

    def t_bf16(x):  # [B,S,D] f32 -> [B,D,S] bf16 contiguous
        return np.ascontiguousarray(
            np.asarray(x, f32).astype(bf).transpose(0, 2, 1)
        )

    qT = t_bf16(q)
    kT = t_bf16(k)
    vT = t_bf16(v)
    Wq_b = np.asarray(Wq, f32).astype(bf)
    Wk_b = np.asarray(Wk, f32).astype(bf)
    Wv_b = np.asarray(Wv, f32).astype(bf)
    Wo_b = np.asarray(Wo, f32).astype(bf)
    bq2 = np.ascontiguousarray(
        (np.asarray(bq, f32) * np.float32(SCALE)).reshape(KC, 128).T
    )
    bk2 = np.ascontiguousarray(np.asarray(bk, f32).reshape(KC, 128).T)
    bv1 = np.ascontiguousarray(np.asarray(bv, f32)).astype(bf)
    bo1 = np.ascontiguousarray(np.asarray(bo, f32)).astype(bf)

    in_maps = []
    for c in range(N_CORES):
        sl = slice(c * NB, (c + 1) * NB)
        in_maps.append({
            "qT": qT[sl], "kT": kT[sl], "vT": vT[sl],
            "Wq": Wq_b, "Wk": Wk_b, "Wv": Wv_b, "Wo": Wo_b,
            "bq": bq2, "bk": bk2, "bv": bv1, "bo": bo1,
        })
    return in_maps


def kernel(q, k, v, Wq, bq, Wk, bk, Wv, bv, Wo, bo):
    nc = _get_program()
    in_maps = prepare_in_maps(q, k, v, Wq, bq, Wk, bk, Wv, bv, Wo, bo)
    res = run_bass_kernel_spmd(nc, in_maps, core_ids=list(range(N_CORES)))
    out = np.concatenate([res.results[c]["out"] for c in range(N_CORES)], axis=0)
    return out.astype(np.float32)


# revision 12
# speedup vs baseline: 1.0515x; 1.0037x over previous
"""CrossAttention (single-head) Trainium2 kernel, 8-core data-parallel.

Full inputs in, full output out. Internally: batch 16 is sharded 2-per-core
across 8 NeuronCores; each core runs the whole attention layer for its two
batches in bf16 (f32 PSUM accumulation), with activations kept in transposed
[d, s] layout so every matmul contracts over the partition dim without any
on-chip transposes of large tensors.

v2 perf changes vs baseline:
- All multi-tile loads (weights, per-block activations) are single-trigger
  3D-AP DMAs: fewer Sync-engine trigger slots, less HWDGE FIFO serialization.
- DMA emission order puts batch-0 K-projection inputs (kin, Wk halves, bk)
  first so the first matmul starts ~9us in instead of ~31us.
- Softmax denominators: the 16 exp tiles are pair-folded on the Vector
  engine down to 2 accumulators, so the partition-sum ones-matmul chain is
  2 matmuls per block instead of 16 (saves ~24us of PE time).
"""

import sys

sys.path.insert(0, "/opt/trn_rl_repo")

import numpy as np
import ml_dtypes

import concourse.bass as bass
import concourse.mybir as mybir
import concourse.tile as tile
from concourse.bass_utils import run_bass_kernel_spmd

BF16 = mybir.dt.bfloat16
F32 = mybir.dt.float32
AF = mybir.ActivationFunctionType

N_CORES = 8
B, S, D = 16, 2048, 1024
NB = B // N_CORES          # batches per core
KC = D // 128              # 8 chunks of 128 along d
ST = S // 128              # 16 tiles of 128 along s
NBLK = S // 512            # 4 blocks of 512 along s
SCALE = 1.0 / np.sqrt(np.float32(D))  # 1/32


def _split_waits(nc, limit=1):
    """Walrus in this container allows at most one sync wait per instruction:
    hoist excess waits onto NoOp carriers inserted just before."""
    n_new = 0
    for f in nc.m.functions:
        for bb in f.blocks:
            new_insts = []
            for inst in bb.instructions:
                si = inst.sync_info
                waits = list(si.on_wait) if si and si.on_wait else []
                if len(waits) > limit:
                    excess, keep = waits[:-limit], waits[-limit:]
                    for i in range(0, len(excess), limit):
                        chunk = excess[i:i + limit]
                        nop = mybir.InstNoOp(
                            name=f"{inst.name}-ws-{n_new}",
                            ins=[], outs=[],
                            sync_info=mybir.SyncInfo(on_wait=chunk, on_update=[]),
                        )
                        nop.engine = inst.engine
                        new_insts.append(nop)
                        n_new += 1
                    si.on_wait = keep
                new_insts.append(inst)
            bb.instructions[:] = new_insts
    return n_new


def _strip_dead_pe_updates(nc):
    """Drop PE sem increments nobody waits on (Tile emits one per matmul;
    only group-stop indices are ever waited). Renumber wait thresholds by
    rank among kept updates — release timing is identical, PE saves ~26ns
    per dropped serialized EVT_SEM write. Straight-line programs only."""
    pe = mybir.EngineType.PE
    insts = [i for f in nc.m.functions for bb in f.blocks for i in bb.instructions]
    upd_by_sem, wait_by_sem, bad = {}, {}, set()
    for inst in insts:
        si = inst.sync_info
        if not si:
            continue
        for u in (si.on_update or []):
            if u.sync_type != "semaphore":
                continue
            if inst.engine != pe or u.update_mode != "sem-inc" or u.update_value != 1:
                bad.add(u.id)
            upd_by_sem.setdefault(u.id, []).append((inst, u))
        for w in (si.on_wait or []):
            if w.sync_type != "semaphore":
                continue
            if w.wait_mode != "sem-ge-imm" or w.wait_reg is not None:
                bad.add(w.id)
            wait_by_sem.setdefault(w.id, []).append(w)
    n_drop = 0
    for sem_id, ups in upd_by_sem.items():
        if sem_id in bad or sem_id not in wait_by_sem or len(ups) < 16:
            continue
        waited = sorted({w.wait_value for w in wait_by_sem[sem_id]})
        if not waited or waited[-1] > len(ups) or waited[0] < 1:
            continue
        keep = set(waited)
        rank = {t: k + 1 for k, t in enumerate(waited)}
        for idx, (inst, u) in enumerate(ups, start=1):
            if idx not in keep:
                inst.sync_info.on_update = [
                    x for x in inst.sync_info.on_update if x is not u
                ]
                n_drop += 1
        for w in wait_by_sem[sem_id]:
            w.wait_value = rank[w.wait_value]
    return n_drop


def build_program(reps=1):
    nc = bass.Bass()

    qT_d = nc.declare_dram_parameter("qT", [NB, D, S], BF16, isOutput=False)
    kT_d = nc.declare_dram_parameter("kT", [NB, D, S], BF16, isOutput=False)
    vT_d = nc.declare_dram_parameter("vT", [NB, D, S], BF16, isOutput=False)
    Wq_d = nc.declare_dram_parameter("Wq", [D, D], BF16, isOutput=False)
    Wk_d = nc.declare_dram_parameter("Wk", [D, D], BF16, isOutput=False)
    Wv_d = nc.declare_dram_parameter("Wv", [D, D], BF16, isOutput=False)
    Wo_d = nc.declare_dram_parameter("Wo", [D, D], BF16, isOutput=False)
    # bq pre-scaled by 1/32 and reshaped [128, KC] host-side; bk likewise unscaled
    bq_d = nc.declare_dram_parameter("bq", [128, KC], F32, isOutput=False)
    bk_d = nc.declare_dram_parameter("bk", [128, KC], F32, isOutput=False)
    bv_d = nc.declare_dram_parameter("bv", [D], BF16, isOutput=False)
    bo_d = nc.declare_dram_parameter("bo", [D], BF16, isOutput=False)
    out_d = nc.declare_dram_parameter("out", [NB, S, D], F32, isOutput=True)

    def w_ap(w_d, col0, ncol):
        """[D, D] weight -> SBUF [128, KC, ncol] chunk-major AP (cols
        col0:col0+ncol of every 128-row chunk) in one DMA."""
        ap = w_d[:]
        return bass.AP(
            tensor=ap.tensor, offset=ap.offset + col0,
            ap=[[D, 128], [128 * D, KC], [1, ncol]],
        )

    def x_ap(x_d, b, s0, ncol):
        """[NB, D, S] activation -> SBUF [128, KC, ncol] chunk-major AP."""
        ap = x_d[:]
        return bass.AP(
            tensor=ap.tensor, offset=ap.offset + b * D * S + s0,
            ap=[[S, 128], [128 * S, KC], [1, ncol]],
        )

    def bcast_ap(v_d):
        ap = v_d[:]
        return bass.AP(tensor=ap.tensor, offset=ap.offset, ap=[[0, 128]] + ap.ap)

    from contextlib import ExitStack
    with tile.TileContext(nc) as tc:
        with ExitStack() as _stk:
            _p = lambda **kw: _stk.enter_context(tc.tile_pool(**kw))
            wqopool = _p(name="wqo", bufs=1)
            wkvpool = _p(name="wkv", bufs=2)
            inpool = _p(name="inp", bufs=3)
            kpool = _p(name="keyT", bufs=8)
            vpool = _p(name="value", bufs=1)
            qpool = _p(name="queryT", bufs=1)
            epool = _p(name="expT", bufs=1)
            fpool = _p(name="fold", bufs=2)
            upool = _p(name="UT", bufs=1)
            opool = _p(name="outb", bufs=2)
            sumpool = _p(name="sums", bufs=2)
            rpool = _p(name="rpool", bufs=2)
            cpool = _p(name="const", bufs=1)
            pspool = _p(name="ps", bufs=5, space="PSUM")
            ps1pool = _p(name="ps1", bufs=1, space="PSUM")
            psrpool = _p(name="psr", bufs=2, space="PSUM")

            # constants (cheap memsets; no DMA)
            ones = cpool.tile([128, 1], BF16, tag="ones")
            nc.vector.memset(ones[:], 1.0)
            ident = cpool.tile([1, 1], F32, tag="ident")
            nc.vector.memset(ident[:], 1.0)

            # ~6us of dummy matmuls fill the initial DMA wait and warm the
            # PE clock gate (HAM) so the real stream starts at 2.4 GHz
            wtile = cpool.tile([128, 128], BF16, tag="warm")
            nc.vector.memset(wtile[:], 0.0)
            warm_ps = psrpool.tile([1, 128], F32, tag="psr", name="warm")
            for _ in range(60):
                nc.tensor.matmul(warm_ps[:], ones[:, 0:1], wtile[:],
                                 start=True, stop=True)

            # ---- batch-0 critical-path DMAs first: kin(sblk0), Wk, bk ----
            # Split so the first chain's first-half accumulation (kin chunks
            # 0-3 x Wk chunks 0-3) can start after ~1MB of transfer; subtile
            # deps gate each matmul on just the DMA covering its region.
            kin0 = inpool.tile([128, KC, 512], BF16, tag="inp", name="kin0")
            wk_b = [None] * NB
            wv_b = [None] * NB
            wk_b[0] = wkvpool.tile([128, KC, D], BF16, tag="wkv", name="wk0")

            def half_x_ap(x_d, b, s0, ch0):
                ap = x_d[:]
                return bass.AP(
                    tensor=ap.tensor,
                    offset=ap.offset + b * D * S + ch0 * 128 * S + s0,
                    ap=[[S, 128], [128 * S, KC // 2], [1, 512]],
                )

            def half_w_ap(w_d, col0, ncol, ch0):
                ap = w_d[:]
                return bass.AP(
                    tensor=ap.tensor, offset=ap.offset + ch0 * 128 * D + col0,
                    ap=[[D, 128], [128 * D, KC // 2], [1, ncol]],
                )

            nc.sync.dma_start(out=kin0[:, 0:4, :], in_=half_x_ap(kT_d, 0, 0, 0))
            nc.sync.dma_start(out=wk_b[0][:, 0:4, 0:512], in_=half_w_ap(Wk_d, 0, 512, 0))
            nc.sync.dma_start(out=kin0[:, 4:8, :], in_=half_x_ap(kT_d, 0, 0, 4))
            nc.sync.dma_start(out=wk_b[0][:, 4:8, 0:512], in_=half_w_ap(Wk_d, 0, 512, 4))
            nc.sync.dma_start(out=wk_b[0][:, :, 512:D], in_=w_ap(Wk_d, 512, 512))
            bk_sb = cpool.tile([128, KC], F32, tag="bk")
            nc.sync.dma_start(out=bk_sb[:], in_=bk_d[:])

            # deferred-load tiles (DMAs emitted mid-stream below)
            bq_sb = cpool.tile([128, KC], F32, tag="bq")
            bv_sb = cpool.tile([128, D], BF16, tag="bv")
            bo_sb = cpool.tile([128, D], BF16, tag="bo")
            wq = wqopool.tile([128, KC, D], BF16, tag="wq")
            wo = wqopool.tile([128, KC, D], BF16, tag="wo")

            import contextlib
            loop_ctx = tc.For_i(0, reps, 1) if reps > 1 else contextlib.nullcontext()
            with loop_ctx:
              for b in range(NB):
                  if b > 0:
                      wk_b[b] = wkvpool.tile([128, KC, D], BF16, tag="wkv",
                                             name=f"wk{b}")
                      nc.sync.dma_start(out=wk_b[b][:], in_=w_ap(Wk_d, 0, D))
                  wk = wk_b[b]

                  # ---------------- keyT[d, s] = Wk.T @ kT (+bk) ----------------
                  keyT = [kpool.tile([128, S], BF16, tag="keyT", name=f"keyT{i}")
                          for i in range(KC)]
                  for s in range(NBLK):
                      if b == 0 and s == 0:
                          kin = kin0
                      else:
                          kin = inpool.tile([128, KC, 512], BF16, tag="inp",
                                            name=f"kin{s}")
                          nc.sync.dma_start(out=kin[:], in_=x_ap(kT_d, b, s * 512, 512))
                      for do in range(KC):
                          psum = pspool.tile([128, 512], F32, tag="ps")
                          for i in range(KC):
                              nc.tensor.matmul(
                                  psum[:], wk[:, i, do * 128:(do + 1) * 128],
                                  kin[:, i, :],
                                  start=(i == 0), stop=(i == KC - 1),
                              )
                          nc.vector.tensor_scalar_add(
                              keyT[do][:, s * 512:(s + 1) * 512], psum[:],
                              bk_sb[:, do:do + 1],
                          )
                      if b == 0 and s == 2:
                          # Wv + bv arrive during remaining K-proj compute
                          # (emitted after kin1/kin2 so the 4MB transfer
                          # doesn't delay the K-proj input stream)
                          wv_b[0] = wkvpool.tile([128, KC, D], BF16, tag="wkv",
                                                 name="wv0")
                          nc.sync.dma_start(out=wv_b[0][:], in_=w_ap(Wv_d, 0, D))
                          nc.sync.dma_start(out=bv_sb[:], in_=bcast_ap(bv_d))

                  if b > 0:
                      wv_b[b] = wkvpool.tile([128, KC, D], BF16, tag="wkv",
                                             name=f"wv{b}")
                      nc.sync.dma_start(out=wv_b[b][:], in_=w_ap(Wv_d, 0, D))
                  wv = wv_b[b]

                  # ---------------- value[s, d] = vT.T @ Wv (+bv) ----------------
                  val = vpool.tile([128, ST, D], BF16, tag="value")
                  for s in range(NBLK):
                      vin = inpool.tile([128, KC, 512], BF16, tag="inp",
                                        name=f"vin{s}")
                      nc.sync.dma_start(out=vin[:], in_=x_ap(vT_d, b, s * 512, 512))
                      for tt in range(4):
                          t16 = s * 4 + tt
                          for n in range(2):
                              psum = pspool.tile([128, 512], F32, tag="ps")
                              for i in range(KC):
                                  nc.tensor.matmul(
                                      psum[:],
                                      vin[:, i, tt * 128:(tt + 1) * 128],
                                      wv[:, i, n * 512:(n + 1) * 512],
                                      start=(i == 0), stop=(i == KC - 1),
                                  )
                              nc.vector.tensor_add(
                                  val[:, t16, n * 512:(n + 1) * 512], psum[:],
                                  bv_sb[:, n * 512:(n + 1) * 512],
                              )
                      if b == 0 and s == 0:
                          # Wq/bq land before the first qry block; Wo/bo are
                          # only read in the final phase, much later
                          nc.sync.dma_start(out=bq_sb[:], in_=bq_d[:])
                          nc.sync.dma_start(out=wq[:], in_=w_ap(Wq_d, 0, D))
                      if b == 0 and s == 2:
                          nc.sync.dma_start(out=wo[:], in_=w_ap(Wo_d, 0, D))
                          nc.sync.dma_start(out=bo_sb[:], in_=bcast_ap(bo_d))

                  # ---------------- per 512-wide sq block ----------------
                  for blk in range(NBLK):
                      # queryT block [d, 512] = Wq.T @ qT_blk, scaled 1/32 (+bq/32)
                      qin = inpool.tile([128, KC, 512], BF16, tag="inp",
                                        name=f"qin{blk}")
                      nc.sync.dma_start(out=qin[:], in_=x_ap(qT_d, b, blk * 512, 512))
                      qry = qpool.tile([128, KC, 512], BF16, tag="queryT")
                      for do in range(KC):
                          psum = pspool.tile([128, 512], F32, tag="ps")
                          for i in range(KC):
                              nc.tensor.matmul(
                                  psum[:], wq[:, i, do * 128:(do + 1) * 128],
                                  qin[:, i, :],
                                  start=(i == 0), stop=(i == KC - 1),
                              )
                          nc.vector.tensor_scalar(
                              out=qry[:, do, :], in0=psum[:], scalar1=float(SCALE),
                              scalar2=bq_sb[:, do:do + 1],
                              op0=mybir.AluOpType.mult, op1=mybir.AluOpType.add,
                          )

                      # scoresT -> expT, with pairwise DVE fold of exp tiles
                      # into 2 accumulators for the partition-sum
                      exp_blk = epool.tile([128, ST, 512], BF16, tag="expT")
                      facc = [
                          fpool.tile([128, 512], BF16, tag="fold", name="facc0"),
                          fpool.tile([128, 512], BF16, tag="fold", name="facc1"),
                      ]
                      for t16 in range(ST):
                          psum = pspool.tile([128, 512], F32, tag="ps")
                          for i in range(KC):
                              nc.tensor.matmul(
                                  psum[:],
                                  keyT[i][:, t16 * 128:(t16 + 1) * 128],
                                  qry[:, i, :],
                                  start=(i == 0), stop=(i == KC - 1),
                              )
                          nc.scalar.activation(exp_blk[:, t16, :], psum[:], AF.Exp)
                          half = t16 // 8
                          if t16 % 8 == 1:
                              nc.vector.tensor_add(
                                  facc[half][:], exp_blk[:, t16 - 1, :],
                                  exp_blk[:, t16, :],
                              )
                          elif t16 % 8 > 1:
                              nc.vector.tensor_add(
                                  facc[half][:], facc[half][:],
                                  exp_blk[:, t16, :],
                              )

                      # column sums over all sk (partition dim): 2 ones-matmuls
                      sums_ps = ps1pool.tile([1, 512], F32, tag="ps1")
                      nc.tensor.matmul(sums_ps[:], ones[:], facc[0][:],
                                       start=True, stop=False)
                      nc.tensor.matmul(sums_ps[:], ones[:], facc[1][:],
                                       start=False, stop=True)
                      sums_sb = sumpool.tile([1, 512], F32, tag="sums")
                      nc.vector.tensor_copy(sums_sb[:], sums_ps[:])

                      # r = 1/sums as per-partition scalars, via [1,128] PE
                      # transpose; emitted before UT so its PE<->DVE chain is
                      # hidden under the UT matmul stream
                      r_sb = rpool.tile([128, 4], F32, tag="r")
                      for m in range(4):
                          pr = psrpool.tile([128, 1], F32, tag="psr")
                          nc.tensor.transpose(
                              pr[:], sums_sb[0:1, m * 128:(m + 1) * 128], ident[:]
                          )
                          nc.vector.reciprocal(r_sb[:, m:m + 1], pr[:])

                      # UT block [d, 512] = value.T @ expT
                      ut = upool.tile([128, KC, 512], BF16, tag="UT")
                      for j in range(KC):
                          psum = pspool.tile([128, 512], F32, tag="ps")
                          for t16 in range(ST):
                              nc.tensor.matmul(
                                  psum[:],
                                  val[:, t16, j * 128:(j + 1) * 128],
                                  exp_blk[:, t16, :],
                                  start=(t16 == 0), stop=(t16 == ST - 1),
                              )
                          nc.vector.tensor_copy(ut[:, j, :], psum[:])

                      # final block: out[sq, d] = (UT.T @ Wo) * r + bo
                      for m in range(4):
                          ob = opool.tile([128, D], F32, tag="outb")
                          sq = blk * 512 + m * 128
                          # split the very last store per 512-half so the
                          # first half ships while the second computes
                          last = (b == NB - 1) and (blk == NBLK - 1) and (m == 3)
                          for n in range(2):
                              psum = pspool.tile([128, 512], F32, tag="ps")
                              for j in range(KC):
                                  nc.tensor.matmul(
                                      psum[:],
                                      ut[:, j, m * 128:(m + 1) * 128],
                                      wo[:, j, n * 512:(n + 1) * 512],
                                      start=(j == 0), stop=(j == KC - 1),
                                  )
                              # ob = (psum * r) + bo in one fused DVE op;
                              # the very last half goes in 256-wide pieces so
                              # compute/store pipeline to the end
                              pieces = 2 if (last and n == 1) else 1
                              for p in range(pieces):
                                  w = 512 // pieces
                                  c0 = n * 512 + p * w
                                  nc.vector.scalar_tensor_tensor(
                                      out=ob[:, c0:c0 + w],
                                      in0=psum[:, p * w:(p + 1) * w],
                                      scalar=r_sb[:, m:m + 1],
                                      in1=bo_sb[:, c0:c0 + w],
                                      op0=mybir.AluOpType.mult,
                                      op1=mybir.AluOpType.add,
                                  )
                                  if last:
                                      nc.sync.dma_start(
                                          out=out_d[b, sq:sq + 128, c0:c0 + w],
                                          in_=ob[:, c0:c0 + w],
                                      )
                          if not last:
                              nc.sync.dma_start(out=out_d[b, sq:sq + 128, :], in_=ob[:])

    if reps == 1:
        _strip_dead_pe_updates(nc)
    _split_waits(nc)
    return nc


_PROGRAM = None


def _get_program():
    global _PROGRAM
    if _PROGRAM is None:
        _PROGRAM = build_program()
    return _PROGRAM


def prepare_in_maps(q, k, v, Wq, bq, Wk, bk, Wv, bv, Wo, bo):
    bf = ml_dtypes.bfloat16
    f32 = np.float32

    def t_bf16(x):  # [B,S,D] f32 -> [B,D,S] bf16 contiguous
        return np.ascontiguousarray(
            np.asarray(x, f32).astype(bf).transpose(0, 2, 1)
        )

    qT = t_bf16(q)
    kT = t_bf16(k)
    vT = t_bf16(v)
    Wq_b = np.asarray(Wq, f32).astype(bf)
    Wk_b = np.asarray(Wk, f32).astype(bf)
    Wv_b = np.asarray(Wv, f32).astype(bf)
    Wo_b = np.asarray(Wo, f32).astype(bf)
    bq2 = np.ascontiguousarray(
        (np.asarray(bq, f32) * np.float32(SCALE)).reshape(KC, 128).T
    )
    bk2 = np.ascontiguousarray(np.asarray(bk, f32).reshape(KC, 128).T)
    bv1 = np.ascontiguousarray(np.asarray(bv, f32)).astype(bf)
    bo1 = np.ascontiguousarray(np.asarray(bo, f32)).astype(bf)

    in_maps = []
    for c in range(N_CORES):
        sl = slice(c * NB, (c + 1) * NB)
        in_maps.append({
            "qT": qT[sl], "kT": kT[sl], "vT": vT[sl],
            "Wq": Wq_b, "Wk": Wk_b, "Wv": Wv_b, "Wo": Wo_b,
            "bq": bq2, "bk": bk2, "bv": bv1, "bo": bo1,
        })
    return in_maps


def kernel(q, k, v, Wq, bq, Wk, bk, Wv, bv, Wo, bo):
    nc = _get_program()
    in_maps = prepare_in_maps(q, k, v, Wq, bq, Wk, bk, Wv, bv, Wo, bo)
    res = run_bass_kernel_spmd(nc, in_maps, core_ids=list(range(N_CORES)))
    out = np.concatenate([res.results[c]["out"] for c in range(N_CORES)], axis=0)
    return out.astype(np.float32)


# revision 13
# speedup vs baseline: 1.3833x; 1.3156x over previous
"""CrossAttention (single-head) Trainium2 kernel, 8-core data-parallel.

Full inputs in, full output out. Internally: batch 16 is sharded 2-per-core
across 8 NeuronCores; each core runs the whole attention layer for its two
batches in bf16 (f32 PSUM accumulation), with activations kept in transposed
[d, s] layout so every matmul contracts over the partition dim without any
on-chip transposes of large tensors.

Weight fusion (host-side algebra, exact):
  scores = (qWq+bq)(kWk+bk)^T/sqrt(D)
         = q M k^T + rowconst + ck^T   with M = Wq Wk^T/sqrt(D),
           ck = k (Wk bq)/sqrt(D); the per-row term is softmax-invariant,
           ck folds into the Exp activation's per-partition bias.
  out    = attn (vWv+bv) Wo + bo = attn v M2 + b2   with M2 = Wv Wo,
           b2 = bv Wo + bo (attention rows sum to 1).
So the device runs only: qM projection, q M k^T, softmax, attn @ v,
(.) @ M2 — the K and V projections vanish (25% of the FLOPs).

Scheduling: single-trigger 3D-AP DMAs ordered so the first matmul starts
~12us in; ~6us of dummy matmuls warm the PE clock gate (HAM) during the
initial DMA wait; softmax denominators via pairwise DVE folds + 2
ones-matmuls; fused (psum*r + b2) DVE epilogue.
"""

import sys

sys.path.insert(0, "/opt/trn_rl_repo")

import numpy as np
import ml_dtypes

import concourse.bass as bass
import concourse.mybir as mybir
import concourse.tile as tile
from concourse.bass_utils import run_bass_kernel_spmd

BF16 = mybir.dt.bfloat16
F32 = mybir.dt.float32
AF = mybir.ActivationFunctionType

N_CORES = 8
B, S, D = 16, 2048, 1024
NB = B // N_CORES          # batches per core
KC = D // 128              # 8 chunks of 128 along d
ST = S // 128              # 16 tiles of 128 along s
NBLK = S // 512            # 4 blocks of 512 along s
SCALE = 1.0 / np.sqrt(np.float32(D))  # 1/32


def _split_waits(nc, limit=1):
    """Walrus in this container allows at most one sync wait per instruction:
    hoist excess waits onto NoOp carriers inserted just before."""
    n_new = 0
    for f in nc.m.functions:
        for bb in f.blocks:
            new_insts = []
            for inst in bb.instructions:
                si = inst.sync_info
                waits = list(si.on_wait) if si and si.on_wait else []
                if len(waits) > limit:
                    excess, keep = waits[:-limit], waits[-limit:]
                    for i in range(0, len(excess), limit):
                        chunk = excess[i:i + limit]
                        nop = mybir.InstNoOp(
                            name=f"{inst.name}-ws-{n_new}",
                            ins=[], outs=[],
                            sync_info=mybir.SyncInfo(on_wait=chunk, on_update=[]),
                        )
                        nop.engine = inst.engine
                        new_insts.append(nop)
                        n_new += 1
                    si.on_wait = keep
                new_insts.append(inst)
            bb.instructions[:] = new_insts
    return n_new


def _strip_dead_pe_updates(nc):
    """Drop PE sem increments nobody waits on (Tile emits one per matmul;
    only group-stop indices are ever waited). Renumber wait thresholds by
    rank among kept updates — release timing is identical, PE saves ~26ns
    per dropped serialized EVT_SEM write. Straight-line programs only."""
    pe = mybir.EngineType.PE
    insts = [i for f in nc.m.functions for bb in f.blocks for i in bb.instructions]
    upd_by_sem, wait_by_sem, bad = {}, {}, set()
    for inst in insts:
        si = inst.sync_info
        if not si:
            continue
        for u in (si.on_update or []):
            if u.sync_type != "semaphore":
                continue
            if inst.engine != pe or u.update_mode != "sem-inc" or u.update_value != 1:
                bad.add(u.id)
            upd_by_sem.setdefault(u.id, []).append((inst, u))
        for w in (si.on_wait or []):
            if w.sync_type != "semaphore":
                continue
            if w.wait_mode != "sem-ge-imm" or w.wait_reg is not None:
                bad.add(w.id)
            wait_by_sem.setdefault(w.id, []).append(w)
    n_drop = 0
    for sem_id, ups in upd_by_sem.items():
        if sem_id in bad or sem_id not in wait_by_sem or len(ups) < 16:
            continue
        waited = sorted({w.wait_value for w in wait_by_sem[sem_id]})
        if not waited or waited[-1] > len(ups) or waited[0] < 1:
            continue
        keep = set(waited)
        rank = {t: k + 1 for k, t in enumerate(waited)}
        for idx, (inst, u) in enumerate(ups, start=1):
            if idx not in keep:
                inst.sync_info.on_update = [
                    x for x in inst.sync_info.on_update if x is not u
                ]
                n_drop += 1
        for w in wait_by_sem[sem_id]:
            w.wait_value = rank[w.wait_value]
    return n_drop


def build_program(reps=1):
    nc = bass.Bass()

    qT_d = nc.declare_dram_parameter("qT", [NB, D, S], BF16, isOutput=False)
    kT_d = nc.declare_dram_parameter("kT", [NB, D, S], BF16, isOutput=False)
    vR_d = nc.declare_dram_parameter("vR", [NB, S, D], BF16, isOutput=False)
    M_d = nc.declare_dram_parameter("M", [D, D], BF16, isOutput=False)
    M2_d = nc.declare_dram_parameter("M2", [D, D], BF16, isOutput=False)
    ck_d = nc.declare_dram_parameter("ck", [NB, 128, ST], F32, isOutput=False)
    b2_d = nc.declare_dram_parameter("b2", [D], BF16, isOutput=False)
    out_d = nc.declare_dram_parameter("out", [NB, S, D], F32, isOutput=True)

    def w_ap(w_d, col0, ncol, ch0=0, nch=KC):
        """[D, D] weight -> SBUF [128, nch, ncol] chunk-major AP."""
        ap = w_d[:]
        return bass.AP(
            tensor=ap.tensor, offset=ap.offset + ch0 * 128 * D + col0,
            ap=[[D, 128], [128 * D, nch], [1, ncol]],
        )

    def x_ap(x_d, b, s0, ncol, ch0=0, nch=KC):
        """[NB, D, S] activation -> SBUF [128, nch, ncol] chunk-major AP."""
        ap = x_d[:]
        return bass.AP(
            tensor=ap.tensor,
            offset=ap.offset + b * D * S + ch0 * 128 * S + s0,
            ap=[[S, 128], [128 * S, nch], [1, ncol]],
        )

    def v_ap(b):
        """[NB, S, D] raw v -> SBUF [128, ST, D] sk-tile-major AP."""
        ap = vR_d[:]
        return bass.AP(
            tensor=ap.tensor, offset=ap.offset + b * S * D,
            ap=[[D, 128], [128 * D, ST], [1, D]],
        )

    from contextlib import ExitStack
    with tile.TileContext(nc) as tc:
        with ExitStack() as _stk:
            _p = lambda **kw: _stk.enter_context(tc.tile_pool(**kw))
            wqopool = _p(name="wqo", bufs=1)
            kpool = _p(name="keyT", bufs=2)
            vpool = _p(name="value", bufs=1)
            inpool = _p(name="inp", bufs=2)
            qpool = _p(name="queryT", bufs=1)
            epool = _p(name="expT", bufs=1)
            fpool = _p(name="fold", bufs=2)
            upool = _p(name="UT", bufs=1)
            opool = _p(name="outb", bufs=2)
            sumpool = _p(name="sums", bufs=2)
            rpool = _p(name="rpool", bufs=2)
            ckpool = _p(name="ckp", bufs=2)
            cpool = _p(name="const", bufs=1)
            pspool = _p(name="ps", bufs=5, space="PSUM")
            ps1pool = _p(name="ps1", bufs=1, space="PSUM")
            psrpool = _p(name="psr", bufs=2, space="PSUM")

            # constants (cheap memsets; no DMA)
            ones = cpool.tile([128, 1], BF16, tag="ones")
            nc.vector.memset(ones[:], 1.0)
            ident = cpool.tile([1, 1], F32, tag="ident")
            nc.vector.memset(ident[:], 1.0)

            # ~6us of dummy matmuls fill the initial DMA wait and warm the
            # PE clock gate (HAM) so the real stream starts at 2.4 GHz
            wtile = cpool.tile([128, 128], BF16, tag="warm")
            nc.vector.memset(wtile[:], 0.0)
            warm_ps = psrpool.tile([1, 128], F32, tag="psr", name="warm")
            for _ in range(60):
                nc.tensor.matmul(warm_ps[:], ones[:, 0:1], wtile[:],
                                 start=True, stop=True)

            # ---- startup DMA order: qin0 + M (interleaved halves) first ----
            qins = {}

            def ensure_qin(g, split=False):
                if g in qins or g >= NB * NBLK:
                    return
                bb, kk = divmod(g, NBLK)
                t = inpool.tile([128, KC, 512], BF16, tag="inp", name=f"qin{g}")
                if split:
                    nc.sync.dma_start(out=t[:, 0:4, :],
                                      in_=x_ap(qT_d, bb, kk * 512, 512, 0, 4))
                else:
                    nc.sync.dma_start(out=t[:], in_=x_ap(qT_d, bb, kk * 512, 512))
                qins[g] = t

            M_sb = wqopool.tile([128, KC, D], BF16, tag="wq", name="M_sb")
            ensure_qin(0, split=True)
            nc.sync.dma_start(out=M_sb[:, 0:4, 0:512], in_=w_ap(M_d, 0, 512, 0, 4))
            nc.sync.dma_start(out=qins[0][:, 4:8, :], in_=x_ap(qT_d, 0, 0, 512, 4, 4))
            nc.sync.dma_start(out=M_sb[:, 4:8, 0:512], in_=w_ap(M_d, 0, 512, 4, 4))
            nc.sync.dma_start(out=M_sb[:, :, 512:D], in_=w_ap(M_d, 512, 512))

            keyTs, cks = {}, {}

            def load_keyT(bb):
                t = kpool.tile([128, KC, S], BF16, tag="keyT", name=f"keyT{bb}")
                nc.sync.dma_start(out=t[:, :, 0:1024], in_=x_ap(kT_d, bb, 0, 1024))
                nc.sync.dma_start(out=t[:, :, 1024:S], in_=x_ap(kT_d, bb, 1024, 1024))
                keyTs[bb] = t
                c = ckpool.tile([128, ST], F32, tag="ck", name=f"ck{bb}")
                nc.sync.dma_start(out=c[:], in_=ck_d[bb])
                cks[bb] = c

            load_keyT(0)
            val0 = vpool.tile([128, ST, D], BF16, tag="value", name="val0")
            nc.sync.dma_start(out=val0[:], in_=v_ap(0))
            M2_sb = wqopool.tile([128, KC, D], BF16, tag="wo", name="M2_sb")
            nc.sync.dma_start(out=M2_sb[:], in_=w_ap(M2_d, 0, D))
            b2_sb = cpool.tile([128, D], BF16, tag="b2")
            _b2ap = b2_d[:]
            nc.sync.dma_start(
                out=b2_sb[:],
                in_=bass.AP(tensor=_b2ap.tensor, offset=_b2ap.offset,
                            ap=[[0, 128]] + _b2ap.ap),
            )

            import contextlib
            loop_ctx = tc.For_i(0, reps, 1) if reps > 1 else contextlib.nullcontext()
            with loop_ctx:
              for b in range(NB):
                  keyT = keyTs[b]
                  ck_sb = cks[b]
                  if b == 0:
                      val = val0
                  else:
                      val = vpool.tile([128, ST, D], BF16, tag="value",
                                       name=f"val{b}")
                      nc.sync.dma_start(out=val[:], in_=v_ap(b))

                  for blk in range(NBLK):
                      g = b * NBLK + blk
                      ensure_qin(g)
                      qin = qins.pop(g)

                      # queryT block [d, 512] = M.T @ qT_blk (scale folded in M)
                      qry = qpool.tile([128, KC, 512], BF16, tag="queryT")
                      for do in range(KC):
                          psum = pspool.tile([128, 512], F32, tag="ps")
                          for i in range(KC):
                              nc.tensor.matmul(
                                  psum[:], M_sb[:, i, do * 128:(do + 1) * 128],
                                  qin[:, i, :],
                                  start=(i == 0), stop=(i == KC - 1),
                              )
                          nc.vector.tensor_copy(qry[:, do, :], psum[:])
                      ensure_qin(g + 1)
                      if b == 0 and blk == 2:
                          # batch-1 keyT/ck stream in during b0 blk2/blk3
                          load_keyT(1)

                      # scoresT -> expT (with per-key ck bias), plus pairwise
                      # DVE fold of exp tiles into 2 accumulators
                      exp_blk = epool.tile([128, ST, 512], BF16, tag="expT")
                      facc = [
                          fpool.tile([128, 512], BF16, tag="fold", name="facc0"),
                          fpool.tile([128, 512], BF16, tag="fold", name="facc1"),
                      ]
                      for t16 in range(ST):
                          psum = pspool.tile([128, 512], F32, tag="ps")
                          for i in range(KC):
                              nc.tensor.matmul(
                                  psum[:],
                                  keyT[:, i, t16 * 128:(t16 + 1) * 128],
                                  qry[:, i, :],
                                  start=(i == 0), stop=(i == KC - 1),
                              )
                          nc.scalar.activation(exp_blk[:, t16, :], psum[:], AF.Exp,
                                               bias=ck_sb[:, t16:t16 + 1])
                          half = t16 // 8
                          if t16 % 8 == 1:
                              nc.vector.tensor_add(
                                  facc[half][:], exp_blk[:, t16 - 1, :],
                                  exp_blk[:, t16, :],
                              )
                          elif t16 % 8 > 1:
                              nc.vector.tensor_add(
                                  facc[half][:], facc[half][:],
                                  exp_blk[:, t16, :],
                              )

                      # column sums over all sk (partition dim): 2 ones-matmuls
                      sums_ps = ps1pool.tile([1, 512], F32, tag="ps1")
                      nc.tensor.matmul(sums_ps[:], ones[:], facc[0][:],
                                       start=True, stop=False)
                      nc.tensor.matmul(sums_ps[:], ones[:], facc[1][:],
                                       start=False, stop=True)
                      sums_sb = sumpool.tile([1, 512], F32, tag="sums")
                      nc.vector.tensor_copy(sums_sb[:], sums_ps[:])

                      # r = 1/sums as per-partition scalars, via [1,128] PE
                      # transpose; emitted before UT so its PE<->DVE chain is
                      # hidden under the UT matmul stream
                      r_sb = rpool.tile([128, 4], F32, tag="r")
                      for m in range(4):
                          pr = psrpool.tile([128, 1], F32, tag="psr")
                          nc.tensor.transpose(
                              pr[:], sums_sb[0:1, m * 128:(m + 1) * 128], ident[:]
                          )
                          nc.vector.reciprocal(r_sb[:, m:m + 1], pr[:])

                      # UT block [d, 512] = v.T @ expT  (raw v)
                      ut = upool.tile([128, KC, 512], BF16, tag="UT")
                      for j in range(KC):
                          psum = pspool.tile([128, 512], F32, tag="ps")
                          for t16 in range(ST):
                              nc.tensor.matmul(
                                  psum[:],
                                  val[:, t16, j * 128:(j + 1) * 128],
                                  exp_blk[:, t16, :],
                                  start=(t16 == 0), stop=(t16 == ST - 1),
                              )
                          nc.vector.tensor_copy(ut[:, j, :], psum[:])

                      # final block: out[sq, d] = (UT.T @ M2) * r + b2
                      for m in range(4):
                          ob = opool.tile([128, D], F32, tag="outb")
                          sq = blk * 512 + m * 128
                          last = (b == NB - 1) and (blk == NBLK - 1) and (m == 3)
                          for n in range(2):
                              psum = pspool.tile([128, 512], F32, tag="ps")
                              for j in range(KC):
                                  nc.tensor.matmul(
                                      psum[:],
                                      ut[:, j, m * 128:(m + 1) * 128],
                                      M2_sb[:, j, n * 512:(n + 1) * 512],
                                      start=(j == 0), stop=(j == KC - 1),
                                  )
                              # ob = (psum * r) + b2 in one fused DVE op; the
                              # very last half goes in 256-wide pieces so
                              # compute/store pipeline to the end
                              pieces = 2 if (last and n == 1) else 1
                              for p in range(pieces):
                                  w = 512 // pieces
                                  c0 = n * 512 + p * w
                                  nc.vector.scalar_tensor_tensor(
                                      out=ob[:, c0:c0 + w],
                                      in0=psum[:, p * w:(p + 1) * w],
                                      scalar=r_sb[:, m:m + 1],
                                      in1=b2_sb[:, c0:c0 + w],
                                      op0=mybir.AluOpType.mult,
                                      op1=mybir.AluOpType.add,
                                  )
                                  if last:
                                      nc.sync.dma_start(
                                          out=out_d[b, sq:sq + 128, c0:c0 + w],
                                          in_=ob[:, c0:c0 + w],
                                      )
                          if not last:
                              nc.sync.dma_start(out=out_d[b, sq:sq + 128, :], in_=ob[:])

    if reps == 1:
        _strip_dead_pe_updates(nc)
    _split_waits(nc)
    return nc


_PROGRAM = None


def _get_program():
    global _PROGRAM
    if _PROGRAM is None:
        _PROGRAM = build_program()
    return _PROGRAM


def prepare_in_maps(q, k, v, Wq, bq, Wk, bk, Wv, bv, Wo, bo):
    bf = ml_dtypes.bfloat16
    f32 = np.float32

    def t_bf16(x):  # [B,S,D] f32 -> [B,D,S] bf16 contiguous
        return np.ascontiguousarray(
            np.asarray(x, f32).astype(bf).transpose(0, 2, 1)
        )

    qT = t_bf16(q)
    kT = t_bf16(k)
    vR = np.ascontiguousarray(np.asarray(v, f32).astype(bf))

    # fused weights (exact algebra; see module docstring)
    Wq_f = np.asarray(Wq, f32)
    Wk_f = np.asarray(Wk, f32)
    Wv_f = np.asarray(Wv, f32)
    Wo_f = np.asarray(Wo, f32)
    bq_f = np.asarray(bq, f32)
    bv_f = np.asarray(bv, f32)
    bo_f = np.asarray(bo, f32)

    M = ((Wq_f @ Wk_f.T) * np.float32(SCALE)).astype(bf)
    M2 = (Wv_f @ Wo_f).astype(bf)
    b2 = (bv_f @ Wo_f + bo_f).astype(bf)
    w_ck = (Wk_f @ bq_f) * np.float32(SCALE)          # [D]
    # ck[b, p, t] = (k[b] @ w_ck)[t*128 + p]
    ck_full = np.asarray(k, f32) @ w_ck               # [B, S]
    ck_full = np.ascontiguousarray(
        ck_full.reshape(B, ST, 128).transpose(0, 2, 1)
    )                                                 # [B, 128, ST]

    in_maps = []
    for c in range(N_CORES):
        sl = slice(c * NB, (c + 1) * NB)
        in_maps.append({
            "qT": qT[sl], "kT": kT[sl], "vR": vR[sl],
            "M": M, "M2": M2, "b2": b2, "ck": ck_full[sl],
        })
    return in_maps


def kernel(q, k, v, Wq, bq, Wk, bk, Wv, bv, Wo, bo):
    nc = _get_program()
    in_maps = prepare_in_maps(q, k, v, Wq, bq, Wk, bk, Wv, bv, Wo, bo)
    res = run_bass_kernel_spmd(nc, in_maps, core_ids=list(range(N_CORES)))
    out = np.concatenate([res.results[c]["out"] for c in range(N_CORES)], axis=0)
    return out.astype(np.float32)
